# Initial kernel scaffold
#
"""Trainium2 Bass kernel for nn_EnhancedTextAttentionBlock.

Self-contained: takes FULL inputs (as in reference.setup_inputs()), shards
across 8 NeuronCores internally, returns the FULL [2, 256, 48, 48] output.

Sharding: core c handles batch b = c // 4 and query-token block k = c % 4
(576 of the 2304 spatial tokens). K/V (and their layernorm) are computed for
the full token set on every core; the query-side path uses host-sliced
inputs, so a single SPMD program serves all 8 cores with no collectives.

Key algebraic restructurings (exact, not approximations):
- The positional encoding pe depends only on (c, w), so the 3x3 conv output
  has only 3 distinct rows (top / middle / bottom). The conv collapses to
  three 1-D convs along w with kh-summed kernels == matmuls over an im2col
  of a [C, 48] tensor.
- v-projection bias commutes through the softmax-normalized attention:
  attn @ (v + 1 v_b^T) = attn @ v + 1 v_b^T, so v_b folds into an effective
  output bias o_b_eff = o_b + v_b @ o_w.T on the host.
- Softmax denominators ride along as an extra ones-column of v and an extra
  l-transport column of the output projection, landing the per-token 1/l in
  token-major layout where it is a cheap per-partition rescale.
- Softmax max-subtraction is skipped: LN'd activations through 0.02-scale
  weights give |scores| < ~2, where exp() is exactly safe in fp32.
"""
import math
import numpy as np

import concourse.bass as bass
import concourse.tile as tile
from concourse import bacc, mybir
from concourse.bass_utils import run_bass_kernel_spmd

import os as _os
F32 = mybir.dt.float32
_PREC = _os.environ.get("KERNEL_PREC", "tuned")
if _os.environ.get("KERNEL_F32") == "1":
    _PREC = "f32"
# per-stage matmul dtypes: projections, scores, attention-value, out-proj
_R = mybir.dt.float32r
PROJ_DT = F32 if _PREC in ("f32", "pf32", "pof32", "tuned") else _R
VPROJ_DT = PROJ_DT  # v matmul shares kn operand; dtype must match
SCORES_DT = F32 if _PREC in ("f32", "sf32") else _R
AV_DT = F32 if _PREC in ("f32", "af32") else _R
OPROJ_DT = F32 if _PREC in ("f32", "af32", "of32", "pof32", "tuned") else _R
F32R = _R
AF = mybir.ActivationFunctionType
OP = mybir.AluOpType

B, C, H, W, T = 2, 256, 48, 48, 512
NH, HD = 8, 32
S = H * W              # 2304 tokens
NQ = S // 4            # 576 q tokens per core
SCALE = HD ** -0.5
IT = 288               # i-tile (two per q block)
MC = 96                # epilogue chunk
EPS = 1e-5

# cvecs column indices (c-major [256, 1] vectors packed into one input)
CV_TMB1, CV_L1G, CV_L1B, CV_TMB2, CV_L2GN, CV_L2BN, \
    CV_NQG, CV_NQB, CV_NKVG, CV_NKVB, CV_CONVB, CV_GW = range(12)
# rowvecs (token-major prebroadcast [128, 256] rows)
RV_OB, RV_NOG, RV_NOB = range(3)


def _partition_stats(nc, pools, x_tiles, n_free, nch, eps_sb, ones_sb, tag):
    """Mean/rsqrt(var) across the partition+chunk (256-channel) dim of
    c-major tiles. x_tiles[cc] = AP [128, n_free]. Returns (mu_bc, rs_bc)
    [128, n_free] broadcast tiles."""
    sb, ps = pools
    HALF = 512
    nhalf = (n_free + HALF - 1) // HALF
    mu = sb.tile([1, n_free], F32, tag="st_mu")
    var = sb.tile([1, n_free], F32, tag="st_var")
    for hf in range(nhalf):
        f0 = hf * HALF
        fn = min(HALF, n_free - f0)
        sum_ps = ps.tile([1, HALF], F32, tag="stsum")
        sumsq_ps = ps.tile([1, HALF], F32, tag="stsumsq")
        for cc in range(nch):
            sq = sb.tile([128, HALF], F32, tag="scrA")
            nc.vector.tensor_mul(sq[:, :fn], x_tiles[cc][:, f0:f0 + fn],
                                 x_tiles[cc][:, f0:f0 + fn])
            nc.tensor.matmul(sum_ps[:, :fn], ones_sb[:], x_tiles[cc][:, f0:f0 + fn],
                             start=(cc == 0), stop=(cc == nch - 1))
            nc.tensor.matmul(sumsq_ps[:, :fn], ones_sb[:], sq[:, :fn],
                             start=(cc == 0), stop=(cc == nch - 1))
        nc.vector.tensor_scalar_mul(mu[:, f0:f0 + fn], sum_ps[:, :fn], 1.0 / 256.0)
        nc.vector.tensor_scalar_mul(var[:, f0:f0 + fn], sumsq_ps[:, :fn], 1.0 / 256.0)
    scr = sb.tile([1, n_free], F32, tag="st_scr")
    nc.vector.tensor_mul(scr[:], mu[:], mu[:])
    nc.vector.tensor_sub(var[:], var[:], scr[:])
    scr2 = sb.tile([1, n_free], F32, tag="st_scr")
    nc.scalar.activation(scr2[:], var[:], AF.Ln, bias=eps_sb[:], scale=1.0)
    rs = sb.tile([1, n_free], F32, tag="st_var2")
    nc.scalar.activation(rs[:], scr2[:], AF.Exp, scale=-0.5)
    mu_bc = sb.tile([128, n_free], F32, tag="st_mubc")
    nc.gpsimd.partition_broadcast(mu_bc[:], mu[:])
    rs_bc = sb.tile([128, n_free], F32, tag="st_rsbc")
    nc.gpsimd.partition_broadcast(rs_bc[:], rs[:])
    return mu_bc, rs_bc


def build_bass():
    nc = bacc.Bacc("TRN2", target_bir_lowering=False, debug=False,
                   enable_asserts=True, num_devices=8)
    di = {}

    def inp(name, shape, dt=F32):
        di[name] = nc.dram_tensor(name, shape, dt, kind="ExternalInput")
        return di[name]

    xk = inp("xk", [C, S])
    xq = inp("xq", [C, NQ])
    xqres = inp("xqres", [NQ, C])
    text = inp("text", [T, 1])
    tmw1 = inp("tmw1", [T, C])
    tmw2 = inp("tmw2", [C, C])
    cvecs = inp("cvecs", [C, 12])
    dvecs = inp("dvecs", [C, 2])
    pe = inp("pe", [C, W])
    w3 = inp("w3", [3, 768, C])
    qwT = inp("qwT", [C, C], PROJ_DT)
    kwT = inp("kwT", [C, C], PROJ_DT)
    vwT = inp("vwT", [C, C], VPROJ_DT)
    owx = inp("owx", [128, NH, 258], OPROJ_DT)
    rowvecs = inp("rowvecs", [128, 3, C])
    selmask = inp("selmask", [128, 2, NQ])
    y = nc.dram_tensor("y", [NQ, C], F32, kind="ExternalOutput")

    with tile.TileContext(nc) as tc:
        _build_tile(nc, tc, di, y)
    nc.compile()
    return nc


def _build_tile(nc, tc, di, y):
    with tc.tile_pool(name="cons", bufs=1) as cons, \
         tc.tile_pool(name="dram", bufs=1, space="DRAM") as dram:
        # ---- persistent small tiles ----
        ones_sb = cons.tile([128, 1], F32)
        nc.vector.memset(ones_sb[:], 1.0)
        eps1 = cons.tile([1, 1], F32)
        nc.vector.memset(eps1[:], EPS)
        cv = cons.tile([128, 2, 12], F32)
        nc.sync.dma_start(out=cv, in_=di["cvecs"].rearrange("(c p) v -> p c v", p=128))
        dv = cons.tile([128, 2, 2], F32)
        nc.sync.dma_start(out=dv, in_=di["dvecs"].rearrange("(c p) v -> p c v", p=128))
        pe_sb = cons.tile([128, 2, W], F32)
        nc.sync.dma_start(out=pe_sb, in_=di["pe"].rearrange("(c p) w -> p c w", p=128))
        qw_sb = cons.tile([128, 2, C], PROJ_DT)
        kw_sb = cons.tile([128, 2, C], PROJ_DT)
        vw_sb = cons.tile([128, 2, C], VPROJ_DT)
        ow_sb = cons.tile([128, NH, 258], OPROJ_DT)
        rv_sb = cons.tile([128, 3, C], F32)
        sel_sb = cons.tile([128, 2, NQ], F32)
        nc.sync.dma_start(out=sel_sb, in_=di["selmask"][:, :, :])
        posrow = cons.tile([128, 2, 3, W], F32)   # (cc, rowtype, w)
        dtop = cons.tile([128, 2, W], F32)
        dbot = cons.tile([128, 2, W], F32)
        qn_sb = cons.tile([128, 2, NQ], PROJ_DT)
        v_tok = cons.tile([128, 18, NH, 33], AV_DT)
        out_acc = cons.tile([MC, 6, C], F32)
        eg_sb = cons.tile([MC, 6], F32)          # exp(-gate logits)
        gate_sb = cons.tile([MC, 6], F32)
        xqres_sb = cons.tile([MC, 6, C], F32)
        ktd = dram.tile([4, 32, 2, S], SCORES_DT)     # per-pair kT in DRAM
        qtd = dram.tile([4, 32, 2, NQ], SCORES_DT)

        # ================= Phase A/B/C/D: prologue ==================
        with tc.tile_pool(name="ph", bufs=1) as ph, \
             tc.tile_pool(name="pps", bufs=2, space="PSUM") as pps, \
             tc.tile_pool(name="spps", bufs=1, space="PSUM") as spps:
            # ---- text modulation MLP (c-major) ----
            text_sb = ph.tile([128, 4, 1], F32)
            nc.sync.dma_start(out=text_sb,
                              in_=di["text"].rearrange("(k p) o -> p k o", p=128))
            w1_sb = ph.tile([128, 4, C], F32, tag="scrB")
            nc.sync.dma_start(out=w1_sb,
                              in_=di["tmw1"].rearrange("(k p) d -> p k d", p=128))
            w2_sb = ph.tile([128, 2, C], F32, tag="scrC")
            nc.sync.dma_start(out=w2_sb,
                              in_=di["tmw2"].rearrange("(k p) d -> p k d", p=128))

            def cmajor_mlp_layer(x_col, w_sb, nkc, bias_col, tag):
                # out[c2] = sum_k w_sb[k, c2] x_col[k]  -> [128, 2, 1] + bias
                h_col = ph.tile([128, 2, 1], F32, tag=f"{tag}_h")
                for c2c in range(2):
                    h_ps = pps.tile([128, 1], F32, tag="mlpps")
                    for kc in range(nkc):
                        nc.tensor.matmul(
                            h_ps[:, :], w_sb[:, kc, c2c * 128:(c2c + 1) * 128],
                            x_col[:, kc, :], start=(kc == 0), stop=(kc == nkc - 1))
                    nc.scalar.activation(h_col[:, c2c, :], h_ps[:, :], AF.Identity,
                                         bias=bias_col[:, c2c, :])
                return h_col

            def cmajor_ln_stats(h_col, tag):
                # 256-dim stats of [128, 2, 1] -> broadcast [128, 1] mu, rs
                sum_ps = spps.tile([1, 1], F32, tag="ssum")
                sq_ps = spps.tile([1, 1], F32, tag="ssq")
                hsq = ph.tile([128, 2, 1], F32, tag=f"{tag}_hsq")
                nc.vector.tensor_mul(hsq[:], h_col[:], h_col[:])
                for cc in range(2):
                    nc.tensor.matmul(sum_ps[:, :], ones_sb[:], h_col[:, cc, :],
                                     start=(cc == 0), stop=(cc == 1))
                    nc.tensor.matmul(sq_ps[:, :], ones_sb[:], hsq[:, cc, :],
                                     start=(cc == 0), stop=(cc == 1))
                mu1 = ph.tile([1, 1], F32, tag=f"{tag}_mu1")
                nc.vector.tensor_scalar_mul(mu1[:], sum_ps[:, :], 1.0 / 256.0)
                var1 = ph.tile([1, 1], F32, tag=f"{tag}_var1")
                nc.vector.tensor_scalar_mul(var1[:], sq_ps[:, :], 1.0 / 256.0)
                musq1 = ph.tile([1, 1], F32, tag=f"{tag}_musq1")
                nc.vector.tensor_mul(musq1[:], mu1[:], mu1[:])
                nc.vector.tensor_sub(var1[:], var1[:], musq1[:])
                nc.scalar.activation(var1[:], var1[:], AF.Ln, bias=eps1[:], scale=1.0)
                nc.scalar.activation(var1[:], var1[:], AF.Exp, scale=-0.5)
                mu_b = ph.tile([128, 1], F32, tag=f"{tag}_mub")
                nc.gpsimd.partition_broadcast(mu_b[:], mu1[:])
                rs_b = ph.tile([128, 1], F32, tag=f"{tag}_rsb")
                nc.gpsimd.partition_broadcast(rs_b[:], var1[:])
                return mu_b, rs_b

            h1 = cmajor_mlp_layer(text_sb, w1_sb, 4, cv[:, :, CV_TMB1:CV_TMB1 + 1], "l1")
            mu_b, rs_b = cmajor_ln_stats(h1, "l1")
            h1n = ph.tile([128, 2, 1], F32, tag="h1n")
            for cc in range(2):
                nc.vector.tensor_sub(h1n[:, cc, :], h1[:, cc, :], mu_b[:])
                nc.vector.tensor_mul(h1n[:, cc, :], h1n[:, cc, :], rs_b[:])
                nc.scalar.activation(h1n[:, cc, :], h1n[:, cc, :], AF.Relu,
                                     bias=cv[:, cc, CV_L1B:CV_L1B + 1], scale=cv[:, cc, CV_L1G:CV_L1G + 1])
            h2 = cmajor_mlp_layer(h1n, w2_sb, 2, cv[:, :, CV_TMB2:CV_TMB2 + 1], "l2")
            mu2_b, rs2_b = cmajor_ln_stats(h2, "l2")
            mod = ph.tile([128, 2, 1], F32, tag="mod")
            for cc in range(2):
                nc.vector.tensor_sub(mod[:, cc, :], h2[:, cc, :], mu2_b[:])
                nc.vector.tensor_mul(mod[:, cc, :], mod[:, cc, :], rs2_b[:])
                # exp(-(g*xn + b)) via pre-negated g, b
                nc.scalar.activation(mod[:, cc, :], mod[:, cc, :], AF.Exp,
                                     bias=cv[:, cc, CV_L2BN:CV_L2BN + 1], scale=cv[:, cc, CV_L2GN:CV_L2GN + 1])
                nc.vector.tensor_scalar(mod[:, cc, :], mod[:, cc, :], 1.0, None, OP.add)
                nc.vector.reciprocal(mod[:, cc, :], mod[:, cc, :])

            # ---- conditional positional rows: 3 distinct conv rows ----
            w3_sb = ph.tile([128, 3, 6, C], F32, tag="bigbuf2")
            nc.sync.dma_start(out=w3_sb,
                              in_=di["w3"].rearrange("t (j p) m -> p t j m", p=128))
            # deferred non-critical loads (behind the prologue-critical DMAs)
            nc.sync.dma_start(out=qw_sb, in_=di["qwT"].rearrange("(c p) d -> p c d", p=128))
            nc.sync.dma_start(out=kw_sb, in_=di["kwT"].rearrange("(c p) d -> p c d", p=128))
            nc.sync.dma_start(out=vw_sb, in_=di["vwT"].rearrange("(c p) d -> p c d", p=128))
            nc.sync.dma_start(out=ow_sb, in_=di["owx"][:, :, :])
            nc.sync.dma_start(out=rv_sb, in_=di["rowvecs"][:, :, :])
            nc.sync.dma_start(out=xqres_sb,
                              in_=di["xqres"].rearrange("(k p) c -> p k c", p=MC))
            inrow = ph.tile([128, 2, W], F32)
            for cc in range(2):
                nc.vector.tensor_scalar_mul(inrow[:, cc, :], pe_sb[:, cc, :],
                                            mod[:, cc, 0:1])
            im2 = ph.tile([128, 6, W], F32, tag="scrC")
            nc.vector.memset(im2[:], 0.0)
            for kw in range(3):
                for cc in range(2):
                    j = kw * 2 + cc
                    if kw == 0:
                        nc.vector.tensor_copy(im2[:, j, 1:W], inrow[:, cc, 0:W - 1])
                    elif kw == 1:
                        nc.vector.tensor_copy(im2[:, j, :], inrow[:, cc, :])
                    else:
                        nc.vector.tensor_copy(im2[:, j, 0:W - 1], inrow[:, cc, 1:W])
            cps = pps.tile([128, 3, 2, W], F32, tag="projps")
            for t in range(3):
                for oc in range(2):
                    for j in range(6):
                        nc.tensor.matmul(cps[:, t, oc, :],
                                         w3_sb[:, t, j, oc * 128:(oc + 1) * 128],
                                         im2[:, j, :],
                                         start=(j == 0), stop=(j == 5))
            for cc in range(2):
                nc.scalar.activation(posrow[:, cc, :, :], cps[:, :, cc, :], AF.Identity,
                                     bias=cv[:, cc, CV_CONVB:CV_CONVB + 1])
                nc.vector.tensor_sub(dtop[:, cc, :], posrow[:, cc, 0, :],
                                     posrow[:, cc, 1, :])
                nc.vector.tensor_sub(dbot[:, cc, :], posrow[:, cc, 2, :],
                                     posrow[:, cc, 1, :])

            # ---- tokens (c-major) ----
            xk_sb = ph.tile([128, 2, S], F32, tag="bigbuf1")
            nc.sync.dma_start(out=xk_sb,
                              in_=di["xk"].rearrange("(c p) s -> p c s", p=128))
            tok = ph.tile([128, 2, S], F32)
            for cc in range(2):
                nc.vector.tensor_add(tok[:, cc, 0:W], xk_sb[:, cc, 0:W],
                                     posrow[:, cc, 0, :])
                mid = posrow[:, cc, 1:2, :].to_broadcast([128, H - 2, W])
                nc.vector.tensor_tensor(
                    tok[:, cc, W:S - W].rearrange("p (h w) -> p h w", w=W),
                    xk_sb[:, cc, W:S - W].rearrange("p (h w) -> p h w", w=W),
                    mid, OP.add)
                nc.vector.tensor_add(tok[:, cc, S - W:S], xk_sb[:, cc, S - W:S],
                                     posrow[:, cc, 2, :])
            xq_sb = ph.tile([128, 2, NQ], F32, tag="scrB")
            nc.sync.dma_start(out=xq_sb,
                              in_=di["xq"].rearrange("(c p) s -> p c s", p=128))
            tokq = ph.tile([128, 2, NQ], F32)
            seltmp = ph.tile([128, NQ], F32, tag="scrA")
            for cc in range(2):
                mid = posrow[:, cc, 1:2, :].to_broadcast([128, NQ // W, W])
                nc.vector.tensor_tensor(
                    tokq[:, cc, :].rearrange("p (h w) -> p h w", w=W),
                    xq_sb[:, cc, :].rearrange("p (h w) -> p h w", w=W),
                    mid, OP.add)
                nc.vector.tensor_tensor(
                    seltmp[:].rearrange("p (h w) -> p h w", w=W),
                    sel_sb[:, 0, :].rearrange("p (h w) -> p h w", w=W),
                    dtop[:, cc, None, :].to_broadcast([128, NQ // W, W]), OP.mult)
                nc.vector.tensor_add(tokq[:, cc, :], tokq[:, cc, :], seltmp[:])
                nc.vector.tensor_tensor(
                    seltmp[:].rearrange("p (h w) -> p h w", w=W),
                    sel_sb[:, 1, :].rearrange("p (h w) -> p h w", w=W),
                    dbot[:, cc, None, :].to_broadcast([128, NQ // W, W]), OP.mult)
                nc.vector.tensor_add(tokq[:, cc, :], tokq[:, cc, :], seltmp[:])

            # ---- layernorms (partition-dim stats) ----
            mu_bc, rs_bc = _partition_stats(
                nc, (ph, spps), [tok[:, cc, :] for cc in range(2)], S, 2,
                eps1, ones_sb, "full")
            kn = ph.tile([128, 2, S], PROJ_DT)
            for cc in range(2):
                t0 = ph.tile([128, S], F32, tag="st_scr")
                nc.vector.tensor_sub(t0[:], tok[:, cc, :], mu_bc[:])
                nc.vector.tensor_mul(t0[:], t0[:], rs_bc[:])
                nc.scalar.activation(kn[:, cc, :], t0[:], AF.Identity,
                                     bias=cv[:, cc, CV_NKVB:CV_NKVB + 1], scale=cv[:, cc, CV_NKVG:CV_NKVG + 1])
            muq_bc, rsq_bc = _partition_stats(
                nc, (ph, spps), [tokq[:, cc, :] for cc in range(2)], NQ, 2,
                eps1, ones_sb, "q")
            for cc in range(2):
                t0 = ph.tile([128, NQ], F32, tag="st_scr")
                nc.vector.tensor_sub(t0[:], tokq[:, cc, :], muq_bc[:])
                nc.vector.tensor_mul(t0[:], t0[:], rsq_bc[:])
                nc.scalar.activation(qn_sb[:, cc, :], t0[:], AF.Identity,
                                     bias=cv[:, cc, CV_NQB:CV_NQB + 1], scale=cv[:, cc, CV_NQG:CV_NQG + 1])

            # ---- projections ----
            STILE = [512, 512, 512, 512, 256]
            kst = ph.tile([128, 2, S], SCORES_DT, tag="bigbuf1")    # reuses xk slot
            for dc in range(2):
                s0 = 0
                for stn in STILE:
                    kp = pps.tile([128, 512], F32, tag="projps")
                    for cc in range(2):
                        nc.tensor.matmul(kp[:, :stn],
                                         kw_sb[:, cc, dc * 128:(dc + 1) * 128],
                                         kn[:, cc, s0:s0 + stn],
                                         start=(cc == 0), stop=(cc == 1))
                    nc.scalar.activation(kst[:, dc, s0:s0 + stn], kp[:, :stn],
                                         AF.Identity, bias=dv[:, dc, 1:2])
                    s0 += stn
            for p in range(4):
                dc = p // 2
                b0 = 64 * (p % 2)
                for hh in range(2):
                    nc.sync.dma_start(out=ktd[p, :, hh, :],
                                      in_=kst[b0 + 32 * hh:b0 + 32 * hh + 32, dc, :])
            qst = ph.tile([128, 2, NQ], SCORES_DT, tag="bigbuf2")
            for dc in range(2):
                for (s0, stn) in ((0, 512), (512, 64)):
                    qp = pps.tile([128, 512], F32, tag="projps")
                    for cc in range(2):
                        nc.tensor.matmul(qp[:, :stn],
                                         qw_sb[:, cc, dc * 128:(dc + 1) * 128],
                                         qn_sb[:, cc, s0:s0 + stn],
                                         start=(cc == 0), stop=(cc == 1))
                    nc.scalar.activation(qst[:, dc, s0:s0 + stn], qp[:, :stn],
                                         AF.Identity, bias=dv[:, dc, 0:1])
            for p in range(4):
                dc = p // 2
                b0 = 64 * (p % 2)
                for hh in range(2):
                    nc.sync.dma_start(out=qtd[p, :, hh, :],
                                      in_=qst[b0 + 32 * hh:b0 + 32 * hh + 32, dc, :])
            # v (token-major) -- bias folded into o_b_eff on host
            nc.vector.tensor_copy(
                v_tok[:, :, :, 32:33],
                ones_sb[:, None, None, :].to_broadcast([128, 18, NH, 1]))
            for sc in range(18):
                vp = pps.tile([128, 512], F32, tag="projps")
                for cc in range(2):
                    kn_v = (kn[:, cc, sc * 128:(sc + 1) * 128]
                            if PROJ_DT == VPROJ_DT else
                            kn.bitcast(VPROJ_DT)[:, cc, sc * 128:(sc + 1) * 128])
                    nc.tensor.matmul(vp[:, 0:C], kn_v,
                                     vw_sb[:, cc, :], start=(cc == 0), stop=(cc == 1))
                nc.vector.tensor_copy(
                    v_tok[:, sc, :, 0:32],
                    vp[:, 0:C].rearrange("p (h d) -> p h d", d=32))
            # gate logits -> exp(-z)
            for ic in range(6):
                gp = pps.tile([MC, 1], F32, tag="mlpps")
                for cc in range(2):
                    nc.tensor.matmul(gp[:, :],
                                     qn_sb.bitcast(F32)[:, cc, ic * MC:(ic + 1) * MC],
                                     cv[:, cc, CV_GW:CV_GW + 1],
                                     start=(cc == 0), stop=(cc == 1))
                nc.scalar.activation(eg_sb[:, ic:ic + 1], gp[:, :], AF.Exp, scale=-1.0)
            nc.vector.tensor_scalar(gate_sb[:], eg_sb[:], 1.0, None, OP.add)
            nc.vector.reciprocal(gate_sb[:], gate_sb[:])

        # ================= attention ==================
        with tc.tile_pool(name="att", bufs=2) as att, \
             tc.tile_pool(name="atte", bufs=3) as atte, \
             tc.tile_pool(name="ps_s", bufs=2, space="PSUM") as ps_s, \
             tc.tile_pool(name="ps_av", bufs=1, space="PSUM") as ps_av, \
             tc.tile_pool(name="ps_o", bufs=2, space="PSUM") as ps_o:
            for p in range(4):
                ktp = att.tile([32, 2, S], SCORES_DT, tag="ktp")
                nc.sync.dma_start(out=ktp, in_=ktd[p, :, :, :])
                qtp = att.tile([32, 2, NQ], SCORES_DT, tag="qtp")
                nc.sync.dma_start(out=qtp, in_=qtd[p, :, :, :])
                for it in range(2):
                    i0 = it * IT
                    av_ps = ps_av.tile([33, 2, 512], F32, tag="avps")
                    for jc in range(18):
                        s_ps = ps_s.tile([128, 2, 512], F32, tag="sps")
                        for hh in range(2):
                            nc.tensor.matmul(
                                s_ps[:, hh, 0:IT],
                                ktp[:, hh, jc * 128:(jc + 1) * 128],
                                qtp[:, hh, i0:i0 + IT], start=True, stop=True)
                        e_sb = atte.tile([128, 2, IT], AV_DT, tag="esb")
                        nc.scalar.activation(e_sb[:, :, :], s_ps[:, :, 0:IT],
                                             AF.Exp, scale=SCALE)
                        for hh in range(2):
                            nc.tensor.matmul(
                                av_ps[:, hh, 0:IT], v_tok[:, jc, 2 * p + hh, :],
                                e_sb[:, hh, :], start=(jc == 0), stop=(jc == 17))
                    av_sb = att.tile([33, 2, IT], OPROJ_DT, tag="avsb")
                    nc.vector.tensor_copy(av_sb[:, :, :], av_ps[:, :, 0:IT])
                    for hh in range(2):
                        h = 2 * p + hh
                        for mc in range(3):
                            ch = it * 3 + mc
                            o_ps = ps_o.tile([MC, 258], F32, tag="ops")
                            nc.tensor.matmul(o_ps[:, :],
                                             av_sb[:, hh, mc * MC:(mc + 1) * MC],
                                             ow_sb[0:33, h, :], start=True, stop=True)
                            r_sb = atte.tile([MC, 1], F32, tag="rsb")
                            nc.vector.reciprocal(r_sb[:], o_ps[:, 256:257])
                            if h == 0:
                                nc.vector.tensor_scalar_mul(
                                    out_acc[:, ch, :], o_ps[:, 0:256], r_sb[:])
                            else:
                                nc.vector.affine_then_add(
                                    out_acc[:, ch, :], o_ps[:, 0:256],
                                    out_acc[:, ch, :], r_sb[:], 0.0)

        # ================= epilogue ==================
        with tc.tile_pool(name="ep", bufs=2) as ep:
            epsm = cons.tile([MC, 1], F32)
            nc.vector.memset(epsm[:], EPS)
            og_all = ep.tile([MC, 6, C], F32, tag="og")
            mv_all = ep.tile([MC, 6, 2], F32, tag="bag")
            for ch in range(6):
                nc.vector.tensor_add(og_all[:, ch, :], out_acc[:, ch, :],
                                     rv_sb[0:MC, RV_OB, :])
                nc.scalar.activation(og_all[:, ch, :], og_all[:, ch, :], AF.Identity,
                                     scale=gate_sb[:, ch:ch + 1])
                stats = ep.tile([MC, nc.vector.BN_STATS_DIM], F32, tag="bst")
                nc.vector.bn_stats(stats[:], og_all[:, ch, :])
                nc.vector.bn_aggr(mv_all[:, ch, :], stats[:])
            # one Ln + one Exp for all 6 chunk rsqrts (avoids table ping-pong)
            rs_all = ep.tile([MC, 6], F32, tag="eprs")
            nc.scalar.activation(rs_all[:], mv_all[:, :, 1], AF.Ln,
                                 bias=epsm[:], scale=1.0)
            nc.scalar.activation(rs_all[:], rs_all[:], AF.Exp, scale=-0.5)
            for ch in range(6):
                t2 = ep.tile([MC, C], F32, tag="ept2")
                nc.vector.tensor_scalar(t2[:], og_all[:, ch, :],
                                        mv_all[:, ch, 0:1], None, OP.subtract)
                nc.vector.tensor_scalar_mul(t2[:], t2[:], rs_all[:, ch:ch + 1])
                nc.vector.tensor_mul(t2[:], t2[:], rv_sb[0:MC, RV_NOG, :])
                nc.vector.tensor_add(t2[:], t2[:], rv_sb[0:MC, RV_NOB, :])
                nc.vector.tensor_add(t2[:], t2[:], xqres_sb[:, ch, :])
                nc.sync.dma_start(
                    out=y.rearrange("(k p) c -> p k c", p=MC)[:, ch, :], in_=t2[:])


def _host_inputs(x, text_feature, tm_w1, tm_b1, tm_ln1_g, tm_ln1_b, tm_w2, tm_b2,
                 tm_ln2_g, tm_ln2_b, conv_w, conv_b, q_w, q_b, k_w, k_b, v_w, v_b,
                 o_w, o_b, gate_w, nq_g, nq_b, nkv_g, nkv_b, no_g, no_b):
    f = np.float32
    # pe table (depends only on (c, w); faithful to reference)
    div = np.exp(np.arange(C // 2, dtype=f) * (-math.log(10000.0) / (C // 2)))
    wpos = np.arange(W, dtype=f)
    s = np.sin(wpos[None, :] * div[:, None])
    c = np.cos(wpos[None, :] * div[:, None])
    pe = np.stack([s, c], axis=1).reshape(C, W).astype(f)
    # kh-collapsed conv kernels: top(kh 1,2), mid(all), bot(kh 0,1)
    w3 = np.stack([
        conv_w[:, :, 1, :] + conv_w[:, :, 2, :],
        conv_w.sum(axis=2),
        conv_w[:, :, 0, :] + conv_w[:, :, 1, :],
    ]).astype(f)                                  # [3, Cout, Cin, kw]
    w3 = w3.transpose(0, 3, 2, 1).reshape(3, 768, C)  # [(kw, cin), cout]
    w3 = np.ascontiguousarray(w3, dtype=f)
    cvecs = np.stack([
        tm_b1, tm_ln1_g, tm_ln1_b, tm_b2, -tm_ln2_g, -tm_ln2_b,
        nq_g, nq_b, nkv_g, nkv_b, conv_b, gate_w[0],
    ], axis=1).astype(f)                          # [256, 12]
    dvecs = np.stack([q_b, k_b], axis=1).astype(f)
    owx = np.zeros((128, NH, 258), f)
    for h in range(NH):
        owx[0:32, h, 0:256] = o_w[:, 32 * h:32 * h + 32].T
        owx[32, h, 256] = 1.0
    ob_eff = (o_b + v_b @ o_w.T).astype(f)
    rowvecs = np.broadcast_to(
        np.stack([ob_eff, no_g, no_b])[None, :, :], (128, 3, C)).astype(f)
    rowvecs = np.ascontiguousarray(rowvecs)

    per_core = []
    for core in range(8):
        b, k = core // 4, core % 4
        xb = np.ascontiguousarray(x[b].reshape(C, S), dtype=f)
        xqc = np.ascontiguousarray(xb[:, NQ * k:NQ * (k + 1)])
        sel = np.zeros((128, 2, NQ), f)
        if k == 0:
            sel[:, 0, 0:W] = 1.0
        if k == 3:
            sel[:, 1, NQ - W:NQ] = 1.0
        per_core.append({
            "xk": xb,
            "xq": xqc,
            "xqres": np.ascontiguousarray(xqc.T),
            "text": np.ascontiguousarray(text_feature[b][:, None], dtype=f),
            "tmw1": np.ascontiguousarray(tm_w1.T, dtype=f),
            "tmw2": np.ascontiguousarray(tm_w2.T, dtype=f),
            "cvecs": cvecs, "dvecs": dvecs, "pe": pe, "w3": w3,
            "qwT": np.ascontiguousarray(q_w.T, dtype=f),
            "kwT": np.ascontiguousarray(k_w.T, dtype=f),
            "vwT": np.ascontiguousarray(v_w.T, dtype=f),
            "owx": owx, "rowvecs": rowvecs, "selmask": sel,
        })
    return per_core


_NC_CACHE = {}


def get_nc():
    if "nc" not in _NC_CACHE:
        _NC_CACHE["nc"] = build_bass()
    return _NC_CACHE["nc"]


def kernel(**inputs):
    inputs = {k: np.asarray(v, dtype=np.float32) for k, v in inputs.items()}
    in_maps = _host_inputs(**inputs)
    nc = get_nc()
    res = run_bass_kernel_spmd(nc, in_maps, core_ids=list(range(8)))
    x = inputs["x"]
    out = np.empty((B, C, H, W), np.float32)
    for b in range(B):
        blocks = [res.results[4 * b + k]["y"] for k in range(4)]  # [NQ, C] each
        tok = np.concatenate(blocks, axis=0)                      # [S, C]
        out[b] = tok.T.reshape(C, H, W)
    return out



# revision 18
# speedup vs baseline: 1.1492x; 1.1492x over previous
"""Trainium2 Bass kernel for nn_EnhancedTextAttentionBlock (v2).

Self-contained: takes FULL inputs (as in reference.setup_inputs()), shards
across 8 NeuronCores internally, returns the FULL [2, 256, 48, 48] output.

Sharding: core c handles batch b = c // 4 and query-token block k = c % 4
(576 of the 2304 spatial tokens). K/V are computed for the full token set on
every core; a single SPMD program serves all 8 cores with no collectives.

Algebraic restructurings (exact, not approximations):
- pe depends only on (c, w): the 3x3 conv collapses to three 1-D convs (bf16).
- LayerNorms are FUSED into the projections: with kn = (tok - mu)*rs*g + b,
  kst = rs ⊙ (kwg^T tok - mu ⊗ kwg_sum) where kwg = kw*g.  The mu-correction
  rides as an extra K=1 matmul accumulation chunk; the per-token rs rides as
  the exp()'s per-partition scale (keys) or is folded into qst (queries).
  kn/qn are never materialized.
- The k-projection bias shifts every score of a query by a constant and
  cancels in softmax -> dropped.  v's LN beta and bias commute through the
  softmax-normalized attention -> folded into the output bias on host.
- Softmax denominators ride as a ones-column of v; av is normalized before
  a head-grouped output projection (4 heads packed into K=128).
- Softmax max-subtraction is skipped: |scores| < ~2 in fp32 exp range.
- Channel-wise token stats are computed as matmul COLUMNS (free-dim 1), so
  the whole stats scalar chain runs on [128, nchunk] tiles (~free).
"""
import math
import numpy as np

import concourse.bass as bass
import concourse.tile as tile
from concourse import bacc, mybir
from concourse.bass_utils import run_bass_kernel_spmd

import os as _os
F32 = mybir.dt.float32
BF16 = mybir.dt.bfloat16
R = mybir.dt.float32r
_PREC = _os.environ.get("KERNEL_PREC", "allr")
PROJ_DT = F32 if _PREC == "f32" else R      # q/k/v projections (moving = tok)
SCORES_DT = F32 if _PREC in ("f32", "sf32") else R
AV_DT = F32 if _PREC in ("f32", "af32") else BF16   # v / exp(scores) storage
OPROJ_DT = F32 if _PREC in ("f32", "of32") else R
AF = mybir.ActivationFunctionType
OP = mybir.AluOpType

B, C, H, W, T = 2, 256, 48, 48, 512
NH, HD = 8, 32
S = H * W              # 2304 tokens
NQ = S // 4            # 576 q tokens per core
SCALE = HD ** -0.5
IT = 288               # q-tile (two per q block)
MC = 96                # epilogue chunk
MEGA = 1152            # k-side pipeline chunk (2 per S)
NCH = S // MEGA        # 2 mega chunks
NJM = MEGA // 128      # 9 key chunks per mega
EPS = 1e-5

# cvecs column indices (c-major [256, 1] vectors packed into one input)
CV_TMB1, CV_L1G, CV_L1B, CV_TMB2, CV_L2GN, CV_L2BN, CV_CONVB, CV_QBT = range(8)
# rowvecs (token-major prebroadcast [128, 256] rows)
RV_OB, RV_NOG, RV_VSN = range(3)


def build_bass():
    nc = bacc.Bacc("TRN2", target_bir_lowering=False, debug=False,
                   enable_asserts=True, num_devices=8)
    di = {}

    def inp(name, shape, dt=F32):
        di[name] = nc.dram_tensor(name, shape, dt, kind="ExternalInput")
        return di[name]

    inp("xk", [C, S])
    inp("xq", [C, NQ])
    inp("xqres", [NQ, C])          # xq^T + no_b (host-folded)
    inp("text", [T, 1])
    inp("tmw1", [T, C])
    inp("tmw2", [C, C])
    inp("cvecs", [C, 8])
    inp("pe", [C, W])
    inp("w3b", [3, 768, C], BF16)
    inp("qwg", [C, C], PROJ_DT)    # [c, d] = q_w[d, c] * nq_g[c]
    inp("kwg", [C, C], PROJ_DT)
    inp("vwg", [C, C], PROJ_DT)    # [c, o] = v_w[o, c] * nkv_g[c]
    inp("owg", [128, 2, C], OPROJ_DT)
    inp("rowvecs", [128, 3, C])
    inp("sumrows", [1, 2, 2, 128])  # [0, r, dc, d]: r0 = -kwg_sum, r1 = -qwg_sum
    inp("gwg", [C, 1])
    inp("gvec", [MC, 2])           # bc cols: gwg_sum, gb_total
    inp("selmask", [128, 2, NQ])
    y = nc.dram_tensor("y", [NQ, C], F32, kind="ExternalOutput")

    with tile.TileContext(nc) as tc:
        _build_tile(nc, tc, di, y)
    nc.compile()
    return nc


def _build_tile(nc, tc, di, y):
    with tc.tile_pool(name="cons", bufs=1) as cons:
        # ---- persistent small tiles ----
        ones_sb = cons.tile([128, 1], F32)
        nc.vector.memset(ones_sb[:], 1.0)
        eps1 = cons.tile([1, 1], F32)
        nc.vector.memset(eps1[:], EPS)
        epsc = cons.tile([128, 1], F32)
        nc.vector.memset(epsc[:], EPS)
        epsm = cons.tile([MC, 1], F32)
        nc.vector.memset(epsm[:], EPS)
        cv = cons.tile([128, 2, 8], F32)
        pe_sb = cons.tile([128, 2, W], F32)
        qw_sb = cons.tile([128, 2, C], PROJ_DT)
        kw_sb = cons.tile([128, 2, C], PROJ_DT)
        vw_sb = cons.tile([128, 2, C], PROJ_DT)
        ow_sb = cons.tile([128, 2, C], OPROJ_DT)
        rv_sb = cons.tile([128, 3, C], F32)
        sr_sb = cons.tile([1, 2, 2, 128], F32)
        gw_sb = cons.tile([128, 2, 1], F32)
        gv_sb = cons.tile([MC, 2], F32)
        sel_sb = cons.tile([128, 2, NQ], F32)
        posrow = cons.tile([128, 2, 3, W], F32)   # (cc, rowtype, w)
        dtop = cons.tile([128, 2, W], F32)
        dbot = cons.tile([128, 2, W], F32)
        # big persistent state
        tok = cons.tile([128, 2, S], F32)
        tokq = cons.tile([128, 2, NQ], F32)
        kst = cons.tile([128, 2, S], SCORES_DT)
        qst = cons.tile([128, 2, NQ], SCORES_DT)
        v_tok = cons.tile([128, 18, NH, 33], AV_DT)
        murow = cons.tile([1, S], F32)          # channel-SUM row (256*mu)
        rssc = cons.tile([128, 18], F32)        # rs(key) columns
        qmurow = cons.tile([1, NQ], F32)        # q channel-SUM row
        rsqbc = cons.tile([128, NQ], F32)       # SCALE * rs_q broadcast
        gate_sb = cons.tile([MC, 6], F32)
        xqres_sb = cons.tile([MC, 6, C], F32)

        with tc.tile_pool(name="work", bufs=2) as wk, \
             tc.tile_pool(name="worksm", bufs=3) as wks, \
             tc.tile_pool(name="pro", bufs=1) as pro, \
             tc.tile_pool(name="att2", bufs=2) as att2, \
             tc.tile_pool(name="atte", bufs=3) as atte, \
             tc.tile_pool(name="avn", bufs=2) as avnp, \
             tc.tile_pool(name="ps_s", bufs=2, space="PSUM") as ps_s, \
             tc.tile_pool(name="ps_av", bufs=1, space="PSUM") as ps_av, \
             tc.tile_pool(name="ps_m", bufs=2, space="PSUM") as ps_m:

            # ---- kick off the big input DMAs first ----
            xk_re = di["xk"].rearrange("(c p) s -> p c s", p=128)
            for m in range(NCH):
                nc.sync.dma_start(out=tok[:, :, m * MEGA:(m + 1) * MEGA],
                                  in_=xk_re[:, :, m * MEGA:(m + 1) * MEGA])
            nc.sync.dma_start(out=tokq,
                              in_=di["xq"].rearrange("(c p) s -> p c s", p=128))
            nc.sync.dma_start(out=cv, in_=di["cvecs"].rearrange("(c p) v -> p c v", p=128))
            nc.sync.dma_start(out=pe_sb, in_=di["pe"].rearrange("(c p) w -> p c w", p=128))

            # ================= text modulation MLP (c-major) =================
            text_sb = pro.tile([128, 4, 1], F32, tag="text")
            nc.sync.dma_start(out=text_sb,
                              in_=di["text"].rearrange("(k p) o -> p k o", p=128))
            w1_sb = pro.tile([128, 4, C], F32, tag="w1")
            nc.sync.dma_start(out=w1_sb,
                              in_=di["tmw1"].rearrange("(k p) d -> p k d", p=128))
            w2_sb = pro.tile([128, 2, C], F32, tag="w2")
            nc.sync.dma_start(out=w2_sb,
                              in_=di["tmw2"].rearrange("(k p) d -> p k d", p=128))

            def cmajor_mlp_layer(x_col, w_sb, nkc, bias_col, tag):
                h_col = wks.tile([128, 2, 1], F32, tag=f"{tag}_h")
                for c2c in range(2):
                    h_ps = ps_m.tile([128, 1], F32, tag="ps")
                    for kc in range(nkc):
                        nc.tensor.matmul(
                            h_ps[:, :], w_sb[:, kc, c2c * 128:(c2c + 1) * 128],
                            x_col[:, kc, :], start=(kc == 0), stop=(kc == nkc - 1))
                    nc.scalar.activation(h_col[:, c2c, :], h_ps[:, :], AF.Identity,
                                         bias=bias_col[:, c2c, :])
                return h_col

            def cmajor_ln_stats(h_col, tag):
                sum_ps = ps_m.tile([1, 2], F32, tag="ps")
                hsq = wks.tile([128, 2, 1], F32, tag=f"{tag}_hsq")
                nc.vector.tensor_mul(hsq[:], h_col[:], h_col[:])
                for cc in range(2):
                    nc.tensor.matmul(sum_ps[:, 0:1], ones_sb[:], h_col[:, cc, :],
                                     start=(cc == 0), stop=(cc == 1))
                    nc.tensor.matmul(sum_ps[:, 1:2], ones_sb[:], hsq[:, cc, :],
                                     start=(cc == 0), stop=(cc == 1))
                ms = wks.tile([1, 2], F32, tag=f"{tag}_ms")
                nc.vector.tensor_scalar_mul(ms[:], sum_ps[:, :], 1.0 / 256.0)
                var1 = wks.tile([1, 1], F32, tag=f"{tag}_var1")
                nc.vector.tensor_mul(var1[:], ms[:, 0:1], ms[:, 0:1])
                nc.vector.tensor_sub(var1[:], ms[:, 1:2], var1[:])
                nc.scalar.activation(var1[:], var1[:], AF.Ln, bias=eps1[:], scale=1.0)
                nc.scalar.activation(var1[:], var1[:], AF.Exp, scale=-0.5)
                mu_b = wks.tile([128, 1], F32, tag=f"{tag}_mub")
                nc.gpsimd.partition_broadcast(mu_b[:], ms[:, 0:1])
                rs_b = wks.tile([128, 1], F32, tag=f"{tag}_rsb")
                nc.gpsimd.partition_broadcast(rs_b[:], var1[:])
                return mu_b, rs_b

            h1 = cmajor_mlp_layer(text_sb, w1_sb, 4, cv[:, :, CV_TMB1:CV_TMB1 + 1], "l1")
            mu_b, rs_b = cmajor_ln_stats(h1, "l1")
            h1n = wks.tile([128, 2, 1], F32, tag="h1n")
            for cc in range(2):
                nc.vector.scalar_tensor_tensor(
                    h1n[:, cc, :], h1[:, cc, :], mu_b[:, 0:1], rs_b[:],
                    OP.subtract, OP.mult)
                nc.scalar.activation(h1n[:, cc, :], h1n[:, cc, :], AF.Relu,
                                     bias=cv[:, cc, CV_L1B:CV_L1B + 1],
                                     scale=cv[:, cc, CV_L1G:CV_L1G + 1])
            h2 = cmajor_mlp_layer(h1n, w2_sb, 2, cv[:, :, CV_TMB2:CV_TMB2 + 1], "l2")
            mu2_b, rs2_b = cmajor_ln_stats(h2, "l2")
            mod = wks.tile([128, 2, 1], F32, tag="mod")
            for cc in range(2):
                nc.vector.scalar_tensor_tensor(
                    mod[:, cc, :], h2[:, cc, :], mu2_b[:, 0:1], rs2_b[:],
                    OP.subtract, OP.mult)
                # sigmoid(z) = 1/(1+exp(-z)) via pre-negated g, b
                nc.scalar.activation(mod[:, cc, :], mod[:, cc, :], AF.Exp,
                                     bias=cv[:, cc, CV_L2BN:CV_L2BN + 1],
                                     scale=cv[:, cc, CV_L2GN:CV_L2GN + 1])
                nc.vector.tensor_scalar(mod[:, cc, :], mod[:, cc, :], 1.0, None, OP.add)
                nc.vector.reciprocal(mod[:, cc, :], mod[:, cc, :])

            # ---- conditional positional rows: 3 distinct conv rows (bf16) ----
            w3_sb = pro.tile([128, 3, 6, C], BF16, tag="w3")
            nc.sync.dma_start(out=w3_sb,
                              in_=di["w3b"].rearrange("t (j p) m -> p t j m", p=128))
            # non-critical loads queued behind the prologue-critical DMAs
            nc.sync.dma_start(out=qw_sb, in_=di["qwg"].rearrange("(c p) d -> p c d", p=128))
            nc.sync.dma_start(out=kw_sb, in_=di["kwg"].rearrange("(c p) d -> p c d", p=128))
            nc.sync.dma_start(out=vw_sb, in_=di["vwg"].rearrange("(c p) d -> p c d", p=128))
            nc.sync.dma_start(out=ow_sb, in_=di["owg"][:, :, :])
            nc.sync.dma_start(out=rv_sb, in_=di["rowvecs"][:, :, :])
            nc.sync.dma_start(out=sr_sb, in_=di["sumrows"][:, :, :, :])
            nc.sync.dma_start(out=gw_sb, in_=di["gwg"].rearrange("(c p) o -> p c o", p=128))
            nc.sync.dma_start(out=gv_sb, in_=di["gvec"][:, :])
            nc.sync.dma_start(out=sel_sb, in_=di["selmask"][:, :, :])
            nc.sync.dma_start(out=xqres_sb,
                              in_=di["xqres"].rearrange("(k p) c -> p k c", p=MC))

            inrow = wks.tile([128, 2, W], BF16, tag="inrow")
            for cc in range(2):
                nc.vector.tensor_scalar_mul(inrow[:, cc, :], pe_sb[:, cc, :],
                                            mod[:, cc, 0:1])
            im2 = wks.tile([128, 6, W], BF16, tag="im2")
            nc.vector.memset(im2[:], 0.0)
            for kw in range(3):
                for cc in range(2):
                    j = kw * 2 + cc
                    if kw == 0:
                        nc.vector.tensor_copy(im2[:, j, 1:W], inrow[:, cc, 0:W - 1])
                    elif kw == 1:
                        nc.vector.tensor_copy(im2[:, j, :], inrow[:, cc, :])
                    else:
                        nc.vector.tensor_copy(im2[:, j, 0:W - 1], inrow[:, cc, 1:W])
            cps = ps_m.tile([128, 3, 2, W], F32, tag="ps")
            for t in range(3):
                for oc in range(2):
                    for j in range(6):
                        nc.tensor.matmul(cps[:, t, oc, :],
                                         w3_sb[:, t, j, oc * 128:(oc + 1) * 128],
                                         im2[:, j, :],
                                         start=(j == 0), stop=(j == 5))
            for cc in range(2):
                nc.vector.tensor_scalar(posrow[:, cc, :, :], cps[:, :, cc, :],
                                        cv[:, cc, CV_CONVB:CV_CONVB + 1], None, OP.add)
                nc.vector.tensor_sub(dtop[:, cc, :], posrow[:, cc, 0, :],
                                     posrow[:, cc, 1, :])
                nc.vector.tensor_sub(dbot[:, cc, :], posrow[:, cc, 2, :],
                                     posrow[:, cc, 1, :])

            # ================= q-side: tokens, stats, projections ============
            seltmp = pro.tile([128, NQ], F32, tag="seltmp")
            for cc in range(2):
                mid = posrow[:, cc, 1:2, :].to_broadcast([128, NQ // W, W])
                tq2 = tokq[:, cc, :].rearrange("p (h w) -> p h w", w=W)
                nc.vector.tensor_tensor(tq2, tq2, mid, OP.add)
                nc.vector.tensor_tensor(
                    seltmp[:].rearrange("p (h w) -> p h w", w=W),
                    sel_sb[:, 0, :].rearrange("p (h w) -> p h w", w=W),
                    dtop[:, cc, None, :].to_broadcast([128, NQ // W, W]), OP.mult)
                nc.vector.tensor_add(tokq[:, cc, :], tokq[:, cc, :], seltmp[:])
                nc.vector.tensor_tensor(
                    seltmp[:].rearrange("p (h w) -> p h w", w=W),
                    sel_sb[:, 1, :].rearrange("p (h w) -> p h w", w=W),
                    dbot[:, cc, None, :].to_broadcast([128, NQ // W, W]), OP.mult)
                nc.vector.tensor_add(tokq[:, cc, :], tokq[:, cc, :], seltmp[:])

            # q stats in column form ([96, 6] chunks)
            sqq = pro.tile([128, 2, NQ], F32, tag="sqq")
            nc.vector.tensor_mul(sqq[:], tokq[:], tokq[:])
            scolq = ps_m.tile([MC, 6, 2], F32, tag="ps")
            for ch in range(6):
                for cc in range(2):
                    nc.tensor.matmul(scolq[:, ch, 0:1],
                                     tokq[:, cc, ch * MC:(ch + 1) * MC],
                                     ones_sb[:], start=(cc == 0), stop=(cc == 1))
                    nc.tensor.matmul(scolq[:, ch, 1:2],
                                     sqq[:, cc, ch * MC:(ch + 1) * MC],
                                     ones_sb[:], start=(cc == 0), stop=(cc == 1))
            mrq = wks.tile([MC, 6, 2], F32, tag="mrq")   # [:,:,0]=mu  [:,:,1]=rs
            nc.vector.tensor_scalar_mul(mrq[:], scolq[:], 1.0 / 256.0)
            varq = wks.tile([MC, 6], F32, tag="varq")
            nc.vector.tensor_mul(varq[:], mrq[:, :, 0], mrq[:, :, 0])
            nc.vector.tensor_sub(varq[:], mrq[:, :, 1], varq[:])
            nc.scalar.activation(mrq[:, :, 1], varq[:], AF.Ln, bias=epsm[:], scale=1.0)
            nc.scalar.activation(mrq[:, :, 1], mrq[:, :, 1], AF.Exp, scale=-0.5)
            # q-side row-form sums (for the q-proj mu-correction and rs_q row)
            sqrow = pro.tile([1, NQ], F32, tag="sqrow")
            for half in range(2):
                h0 = half * IT
                mrow_ps = ps_m.tile([1, IT], F32, tag="ps")
                for cc in range(2):
                    nc.tensor.matmul(mrow_ps[:, :], ones_sb[:],
                                     tokq[:, cc, h0:h0 + IT],
                                     start=(cc == 0), stop=(cc == 1))
                nc.vector.tensor_copy(qmurow[:, h0:h0 + IT], mrow_ps[:, :])
                srow_ps = ps_m.tile([1, IT], F32, tag="ps")
                for cc in range(2):
                    nc.tensor.matmul(srow_ps[:, :], ones_sb[:],
                                     sqq[:, cc, h0:h0 + IT],
                                     start=(cc == 0), stop=(cc == 1))
                nc.vector.tensor_copy(sqrow[:, h0:h0 + IT], srow_ps[:, :])
            # rs_q row: rsqrt(sumsq/256 - (sum/256)^2 + eps)
            qsc = pro.tile([1, NQ], F32, tag="qsc")
            nc.vector.tensor_scalar_mul(qsc[:], qmurow[:], 1.0 / 256.0)
            nc.vector.tensor_mul(qsc[:], qsc[:], qsc[:])
            nc.vector.scalar_tensor_tensor(qsc[:], sqrow[:], 1.0 / 256.0, qsc[:],
                                           OP.mult, OP.subtract)
            nc.scalar.activation(qsc[:], qsc[:], AF.Ln, bias=eps1[:], scale=1.0)
            nc.scalar.activation(qsc[:], qsc[:], AF.Exp, scale=-0.5)
            nc.vector.tensor_scalar_mul(qsc[:], qsc[:], SCALE)
            nc.gpsimd.partition_broadcast(rsqbc[:], qsc[:])

            # q projection (fused LN): psum = qwg^T tokq - mu_q x qwg_sum
            for dc in range(2):
                for half in range(2):
                    h0 = half * IT
                    qp = ps_m.tile([128, IT], F32, tag="ps")
                    for cc in range(2):
                        nc.tensor.matmul(qp[:, :],
                                         qw_sb[:, cc, dc * 128:(dc + 1) * 128],
                                         tokq.bitcast(PROJ_DT)[:, cc, h0:h0 + IT],
                                         start=(cc == 0), stop=False)
                    nc.tensor.matmul(qp[:, :], sr_sb[0:1, 1, dc, :],
                                     qmurow[0:1, h0:h0 + IT],
                                     start=False, stop=True)
                    # qst = SCALE*rs_q (.) psum + qb_total x 1
                    tq = att2.tile([128, IT], F32, tag="tq")
                    nc.vector.tensor_mul(tq[:], qp[:, :], rsqbc[:, h0:h0 + IT])
                    nc.vector.tensor_scalar(
                        qst.bitcast(F32)[:, dc, h0:h0 + IT], tq[:],
                        cv[:, dc, CV_QBT:CV_QBT + 1], None, OP.add)

            # gate logits (fused LN), column form [96, 6]
            gcol = ps_m.tile([MC, 6], F32, tag="ps")
            for ch in range(6):
                for cc in range(2):
                    nc.tensor.matmul(gcol[:, ch:ch + 1],
                                     tokq[:, cc, ch * MC:(ch + 1) * MC],
                                     gw_sb[:, cc, :], start=(cc == 0), stop=(cc == 1))
            glog = wks.tile([MC, 6], F32, tag="glog")
            # glog = rs_q .* (raw - mu_q * gwg_sum) + gb_total
            nc.vector.tensor_scalar_mul(glog[:], mrq[:, :, 0], gv_sb[:, 0:1])
            nc.vector.tensor_sub(glog[:], gcol[:, :], glog[:])
            nc.vector.tensor_mul(glog[:], glog[:], mrq[:, :, 1])
            nc.vector.tensor_scalar(glog[:], glog[:], gv_sb[:, 1:2], None, OP.add)
            eg = wks.tile([MC, 6], F32, tag="eg")
            nc.scalar.activation(eg[:], glog[:], AF.Exp, scale=-1.0)
            nc.vector.tensor_scalar(gate_sb[:], eg[:], 1.0, None, OP.add)
            nc.vector.reciprocal(gate_sb[:], gate_sb[:])

            # ================= k-side mega-chunk pipeline ====================
            def kside_mega(m):
                t0 = m * MEGA
                nrows = MEGA // W  # 24
                for cc in range(2):
                    r0 = 0
                    if m == 0:   # image top row
                        nc.vector.tensor_add(tok[:, cc, 0:W], tok[:, cc, 0:W],
                                             posrow[:, cc, 0, :])
                        r0 = 1
                    r1 = nrows
                    if m == NCH - 1:  # image bottom row
                        nc.vector.tensor_add(tok[:, cc, t0 + MEGA - W:t0 + MEGA],
                                             tok[:, cc, t0 + MEGA - W:t0 + MEGA],
                                             posrow[:, cc, 2, :])
                        r1 = nrows - 1
                    a, b = t0 + r0 * W, t0 + r1 * W
                    mid = posrow[:, cc, 1:2, :].to_broadcast([128, r1 - r0, W])
                    tv = tok[:, cc, a:b].rearrange("p (h w) -> p h w", w=W)
                    nc.vector.tensor_tensor(tv, tv, mid, OP.add)
                # stats columns
                sq = pro.tile([128, 2, MEGA], F32, tag="sqk")
                nc.vector.tensor_mul(sq[:], tok[:, :, t0:t0 + MEGA],
                                     tok[:, :, t0:t0 + MEGA])
                scol = ps_m.tile([128, NJM, 2], F32, tag="ps")
                for ch in range(NJM):
                    a = t0 + ch * 128
                    for cc in range(2):
                        nc.tensor.matmul(scol[:, ch, 0:1], tok[:, cc, a:a + 128],
                                         ones_sb[:], start=(cc == 0), stop=(cc == 1))
                        nc.tensor.matmul(scol[:, ch, 1:2],
                                         sq[:, cc, ch * 128:(ch + 1) * 128],
                                         ones_sb[:], start=(cc == 0), stop=(cc == 1))
                mm = wks.tile([128, NJM, 2], F32, tag="mmk")
                nc.vector.tensor_scalar_mul(mm[:], scol[:], 1.0 / 256.0)
                var = wks.tile([128, NJM], F32, tag="vark")
                nc.vector.tensor_mul(var[:], mm[:, :, 0], mm[:, :, 0])
                nc.vector.tensor_sub(var[:], mm[:, :, 1], var[:])
                rsk = rssc[:, m * NJM:(m + 1) * NJM]
                nc.scalar.activation(rsk, var[:], AF.Ln, bias=epsc[:], scale=1.0)
                nc.scalar.activation(rsk, rsk, AF.Exp, scale=-0.5)
                # channel-SUM rows (mu-correction moving operand)
                for quar in range(4):
                    a = t0 + quar * IT
                    mrow_ps = ps_m.tile([1, IT], F32, tag="ps")
                    for cc in range(2):
                        nc.tensor.matmul(mrow_ps[:, :], ones_sb[:],
                                         tok[:, cc, a:a + IT],
                                         start=(cc == 0), stop=(cc == 1))
                    nc.vector.tensor_copy(murow[:, a:a + IT], mrow_ps[:, :])
                # k projection (fused LN, no bias; rs rides in the exp scale)
                for dc in range(2):
                    for quar in range(4):
                        a = t0 + quar * IT
                        kp = ps_m.tile([128, IT], F32, tag="ps")
                        for cc in range(2):
                            nc.tensor.matmul(kp[:, :],
                                             kw_sb[:, cc, dc * 128:(dc + 1) * 128],
                                             tok.bitcast(PROJ_DT)[:, cc, a:a + IT],
                                             start=(cc == 0), stop=False)
                        nc.tensor.matmul(kp[:, :], sr_sb[0:1, 0, dc, :],
                                         murow[0:1, a:a + IT],
                                         start=False, stop=True)
                        nc.vector.tensor_copy(kst.bitcast(F32)[:, dc, a:a + IT],
                                              kp[:, :])
                # v projection (fused LN; rs applied on the psum->sbuf copy)
                for ch in range(NJM):
                    jc = m * NJM + ch
                    a = t0 + ch * 128
                    vp = ps_m.tile([128, C], F32, tag="ps")
                    for cc in range(2):
                        nc.tensor.matmul(vp[:, :],
                                         tok.bitcast(PROJ_DT)[:, cc, a:a + 128],
                                         vw_sb[:, cc, :], start=(cc == 0), stop=False)
                    nc.tensor.matmul(vp[:, :], murow[0:1, a:a + 128],
                                     rv_sb[0:1, RV_VSN, :], start=False, stop=True)
                    nc.vector.tensor_scalar(
                        v_tok[:, jc, :, 0:32], vp[:, :].rearrange(
                            "p (h d) -> p h d", d=32),
                        rssc[:, jc:jc + 1], None, OP.mult)
                nc.vector.tensor_copy(
                    v_tok[:, m * NJM:(m + 1) * NJM, :, 32:33],
                    ones_sb[:, None, None, :].to_broadcast([128, NJM, NH, 1]))

            kside_mega(0)

            # ================= attention + epilogue ==========================
            state = {}

            def attn_block(it, p, m):
                i0 = it * IT
                av_ps = state["av_ps"]
                for ch in range(NJM):
                    jc = m * NJM + ch
                    s_ps = ps_s.tile([128, 2, 512], F32, tag="sps")
                    for hh in range(2):
                        h = 2 * p + hh
                        dc, poff = h // 4, 32 * (h % 4)
                        nc.tensor.matmul(
                            s_ps[:, hh, 0:IT],
                            kst[poff:poff + 32, dc, jc * 128:(jc + 1) * 128],
                            qst[poff:poff + 32, dc, i0:i0 + IT],
                            start=True, stop=True, tile_position=(poff, 0))
                    e_sb = atte.tile([128, 2, IT], AV_DT, tag="esb")
                    nc.scalar.activation(e_sb[:, :, :], s_ps[:, :, 0:IT],
                                         AF.Exp, scale=rssc[:, jc:jc + 1])
                    for hh in range(2):
                        nc.tensor.matmul(
                            av_ps[:, hh, 0:IT], v_tok[:, jc, 2 * p + hh, :],
                            e_sb[:, hh, :],
                            start=(jc == 0), stop=(jc == 17))

            for it in range(2):
                av_n = avnp.tile([128, 2, IT], OPROJ_DT, tag="avn")
                for p in range(4):
                    av_ps = ps_av.tile([33, 2, 512], F32, tag="avps")
                    state["av_ps"] = av_ps
                    attn_block(it, p, 0)
                    if it == 0 and p == 0:
                        kside_mega(1)
                    attn_block(it, p, 1)
                    # normalize: av_n[h] = av[h] * (1 / l[h])
                    av_ps = state["av_ps"]
                    r_sb = att2.tile([1, 2, IT], F32, tag="rsb")
                    nc.vector.reciprocal(r_sb[:], av_ps[32:33, :, 0:IT])
                    r_bc = att2.tile([32, 2, IT], F32, tag="rbc")
                    nc.gpsimd.partition_broadcast(r_bc[:], r_sb[:])
                    for hh in range(2):
                        h = 2 * p + hh
                        g, poff = h // 4, 32 * (h % 4)
                        nc.vector.tensor_tensor(
                            av_n.bitcast(F32)[poff:poff + 32, g, :],
                            av_ps[0:32, hh, 0:IT], r_bc[:, hh, :], OP.mult)
                # output projection + epilogue per 96-token chunk
                for mc in range(3):
                    ch = it * 3 + mc
                    o_ps = ps_m.tile([MC, C], F32, tag="ps")
                    for g in range(2):
                        nc.tensor.matmul(o_ps[:, :],
                                         av_n[:, g, mc * MC:(mc + 1) * MC],
                                         ow_sb[:, g, :], start=(g == 0), stop=(g == 1))
                    og = wk.tile([MC, C], F32, tag="og")
                    nc.vector.tensor_add(og[:], o_ps[:, :], rv_sb[0:MC, RV_OB, :])
                    nc.vector.tensor_scalar_mul(og[:], og[:], gate_sb[:, ch:ch + 1])
                    stats = wks.tile([MC, nc.vector.BN_STATS_DIM], F32, tag="bst")
                    nc.vector.bn_stats(stats[:], og[:])
                    mv = wks.tile([MC, 2], F32, tag="bag")
                    nc.vector.bn_aggr(mv[:], stats[:])
                    rs2 = wks.tile([MC, 1], F32, tag="eprs")
                    nc.scalar.activation(rs2[:], mv[:, 1:2], AF.Ln,
                                         bias=epsm[:], scale=1.0)
                    nc.scalar.activation(rs2[:], rs2[:], AF.Exp, scale=-0.5)
                    rsn = wks.tile([MC, C], F32, tag="rsn")
                    nc.vector.tensor_scalar_mul(rsn[:], rv_sb[0:MC, RV_NOG, :], rs2[:])
                    t2 = wk.tile([MC, C], F32, tag="ept2")
                    nc.vector.scalar_tensor_tensor(
                        t2[:], og[:], mv[:, 0:1], rsn[:], OP.subtract, OP.mult)
                    nc.vector.tensor_add(t2[:], t2[:], xqres_sb[:, ch, :])
                    nc.sync.dma_start(
                        out=y.rearrange("(k p) c -> p k c", p=MC)[:, ch, :], in_=t2[:])


def _host_inputs(x, text_feature, tm_w1, tm_b1, tm_ln1_g, tm_ln1_b, tm_w2, tm_b2,
                 tm_ln2_g, tm_ln2_b, conv_w, conv_b, q_w, q_b, k_w, k_b, v_w, v_b,
                 o_w, o_b, gate_w, nq_g, nq_b, nkv_g, nkv_b, no_g, no_b):
    f = np.float32
    # pe table (depends only on (c, w); faithful to reference)
    div = np.exp(np.arange(C // 2, dtype=f) * (-math.log(10000.0) / (C // 2)))
    wpos = np.arange(W, dtype=f)
    s = np.sin(wpos[None, :] * div[:, None])
    c = np.cos(wpos[None, :] * div[:, None])
    pe = np.stack([s, c], axis=1).reshape(C, W).astype(f)
    # kh-collapsed conv kernels: top(kh 1,2), mid(all), bot(kh 0,1)
    w3 = np.stack([
        conv_w[:, :, 1, :] + conv_w[:, :, 2, :],
        conv_w.sum(axis=2),
        conv_w[:, :, 0, :] + conv_w[:, :, 1, :],
    ]).astype(f)                                  # [3, Cout, Cin, kw]
    w3 = w3.transpose(0, 3, 2, 1).reshape(3, 768, C)  # [(kw, cin), cout]
    import ml_dtypes
    w3b = np.ascontiguousarray(w3).astype(ml_dtypes.bfloat16)

    # LN-fused projection weights
    qwg = np.ascontiguousarray(q_w.T * nq_g[:, None], dtype=f)   # [c, d]
    kwg = np.ascontiguousarray(k_w.T * nkv_g[:, None], dtype=f)
    vwg = np.ascontiguousarray(v_w.T * nkv_g[:, None], dtype=f)  # [c, o]
    qb_total = (q_b + q_w @ nq_b).astype(f)
    # correction rows pair with channel-SUM rows -> fold the 1/256 here
    sumrows = np.stack([-kwg.sum(axis=0) / 256.0, -qwg.sum(axis=0) / 256.0]) \
        .reshape(1, 2, 2, 128).astype(f)
    # v bias (incl LN beta) folds through softmax-normalized attention
    vb_total = (v_b + v_w @ nkv_b).astype(f)
    ob_eff = (o_b + vb_total @ o_w.T).astype(f)
    # gate
    gwg = np.ascontiguousarray((gate_w[0] * nq_g)[:, None], dtype=f)  # [C, 1]
    gvec = np.zeros((MC, 2), f)
    gvec[:, 0] = gwg.sum()
    gvec[:, 1] = gate_w[0] @ nq_b
    # head-grouped output projection: partition 32*(h%4)+d, group h//4
    owg = np.zeros((128, 2, C), f)
    for h in range(NH):
        owg[32 * (h % 4):32 * (h % 4) + 32, h // 4, :] = o_w[:, 32 * h:32 * h + 32].T
    rowvecs = np.zeros((128, 3, C), f)
    rowvecs[:, RV_OB, :] = ob_eff[None, :]
    rowvecs[:, RV_NOG, :] = no_g[None, :]
    rowvecs[0, RV_VSN, :] = -vwg.sum(axis=0) / 256.0
    cvecs = np.stack([
        tm_b1, tm_ln1_g, tm_ln1_b, tm_b2, -tm_ln2_g, -tm_ln2_b, conv_b, qb_total,
    ], axis=1).astype(f)                          # [256, 8]

    per_core = []
    for core in range(8):
        b, k = core // 4, core % 4
        xb = np.ascontiguousarray(x[b].reshape(C, S), dtype=f)
        xqc = np.ascontiguousarray(xb[:, NQ * k:NQ * (k + 1)])
        sel = np.zeros((128, 2, NQ), f)
        if k == 0:
            sel[:, 0, 0:W] = 1.0
        if k == 3:
            sel[:, 1, NQ - W:NQ] = 1.0
        per_core.append({
            "xk": xb,
            "xq": xqc,
            "xqres": np.ascontiguousarray(xqc.T + no_b[None, :]),
            "text": np.ascontiguousarray(text_feature[b][:, None], dtype=f),
            "tmw1": np.ascontiguousarray(tm_w1.T, dtype=f),
            "tmw2": np.ascontiguousarray(tm_w2.T, dtype=f),
            "cvecs": cvecs, "pe": pe, "w3b": w3b,
            "qwg": qwg, "kwg": kwg, "vwg": vwg, "owg": owg,
            "rowvecs": rowvecs, "sumrows": sumrows,
            "gwg": gwg, "gvec": gvec, "selmask": sel,
        })
    return per_core


_NC_CACHE = {}


def get_nc():
    if "nc" not in _NC_CACHE:
        _NC_CACHE["nc"] = build_bass()
    return _NC_CACHE["nc"]


def kernel(**inputs):
    inputs = {k: np.asarray(v, dtype=np.float32) for k, v in inputs.items()}
    in_maps = _host_inputs(**inputs)
    nc = get_nc()
    res = run_bass_kernel_spmd(nc, in_maps, core_ids=list(range(8)))
    x = inputs["x"]
    out = np.empty((B, C, H, W), np.float32)
    for b in range(B):
        blocks = [res.results[4 * b + k]["y"] for k in range(4)]  # [NQ, C] each
        tok = np.concatenate(blocks, axis=0)                      # [S, C]
        out[b] = tok.T.reshape(C, H, W)
    return out


# revision 31
# speedup vs baseline: 1.1613x; 1.0105x over previous
"""Trainium2 Bass kernel for nn_EnhancedTextAttentionBlock (v2).

Self-contained: takes FULL inputs (as in reference.setup_inputs()), shards
across 8 NeuronCores internally, returns the FULL [2, 256, 48, 48] output.

Sharding: core c handles batch b = c // 4 and query-token block k = c % 4
(576 of the 2304 spatial tokens). K/V are computed for the full token set on
every core; a single SPMD program serves all 8 cores with no collectives.

Algebraic restructurings (exact, not approximations):
- pe depends only on (c, w): the 3x3 conv collapses to three 1-D convs (bf16).
- LayerNorms are FUSED into the projections: with kn = (tok - mu)*rs*g + b,
  kst = rs ⊙ (kwg^T tok - mu ⊗ kwg_sum) where kwg = kw*g.  The mu-correction
  rides as an extra K=1 matmul accumulation chunk; the per-token rs rides as
  the exp()'s per-partition scale (keys) or is folded into qst (queries).
  kn/qn are never materialized.
- The k-projection bias shifts every score of a query by a constant and
  cancels in softmax -> dropped.  v's LN beta and bias commute through the
  softmax-normalized attention -> folded into the output bias on host.
- Softmax denominators ride as a ones-column of v; av is normalized before
  a head-grouped output projection (4 heads packed into K=128).
- Softmax max-subtraction is skipped: |scores| < ~2 in fp32 exp range.
- Channel-wise token stats are computed as matmul COLUMNS (free-dim 1), so
  the whole stats scalar chain runs on [128, nchunk] tiles (~free).
"""
import math
import numpy as np

import concourse.bass as bass
import concourse.tile as tile
from concourse import bacc, mybir
from concourse.bass_utils import run_bass_kernel_spmd

import os as _os
F32 = mybir.dt.float32
BF16 = mybir.dt.bfloat16
R = mybir.dt.float32r
_PREC = _os.environ.get("KERNEL_PREC", "allr")
PROJ_DT = F32 if _PREC == "f32" else R      # q/k/v projections (moving = tok)
SCORES_DT = F32 if _PREC in ("f32", "sf32") else BF16
AV_DT = F32 if _PREC in ("f32", "af32") else BF16   # v / exp(scores) storage
OPROJ_DT = F32 if _PREC in ("f32", "of32") else BF16
AF = mybir.ActivationFunctionType
OP = mybir.AluOpType

B, C, H, W, T = 2, 256, 48, 48, 512
NH, HD = 8, 32
S = H * W              # 2304 tokens
NQ = S // 4            # 576 q tokens per core
SCALE = HD ** -0.5
IT = 288               # q-tile (two per q block)
MC = 96                # epilogue chunk
MEGA = 1152            # k-side pipeline chunk (2 per S)
NCH = S // MEGA        # 2 mega chunks
NJM = MEGA // 128      # 9 key chunks per mega
EPS = 1e-5

# cvecs column indices (c-major [256, 1] vectors packed into one input)
CV_TMB1, CV_L1G, CV_L1B, CV_TMB2, CV_L2GN, CV_L2BN, CV_CONVB, CV_QBT = range(8)
# rowvecs (token-major prebroadcast [128, 256] rows)
RV_OB, RV_NOG = range(2)


def build_bass():
    nc = bacc.Bacc("TRN2", target_bir_lowering=False, debug=False,
                   enable_asserts=True, num_devices=8)
    di = {}

    def inp(name, shape, dt=F32):
        di[name] = nc.dram_tensor(name, shape, dt, kind="ExternalInput")
        return di[name]

    inp("xk", [C, S])
    inp("xq", [C, NQ])
    inp("xqres", [NQ, C])          # xq^T + no_b (host-folded)
    inp("text", [T, 1])
    inp("tmw1", [T, C])
    inp("tmw2", [C, C])
    inp("cvecs", [C, 8])
    inp("pe", [C, W])
    inp("w3b", [3, 768, C])
    inp("qwg", [C, C], PROJ_DT)    # [c, d] = q_w[d, c] * nq_g[c]
    inp("kwg", [C, C], PROJ_DT)
    inp("vwg", [C, C], PROJ_DT)    # [c, o] = v_w[o, c] * nkv_g[c]
    inp("owg", [128, 2, C], OPROJ_DT)
    inp("rowvecs", [128, 2, C])
    inp("vsn", [1, C], PROJ_DT)
    inp("sumrows", [1, 2, 2, 128], PROJ_DT)  # [0, r, dc, d]: r0 = -kwg_sum, r1 = -qwg_sum
    inp("gwg", [C, 1])
    inp("gvec", [MC, 2])           # bc cols: gwg_sum, gb_total
    inp("selmask", [128, 2, NQ])
    y = nc.dram_tensor("y", [NQ, C], F32, kind="ExternalOutput")
    if _os.environ.get("KERNEL_DBG") == "1":
        di["dbg_tok"] = nc.dram_tensor("dbg_tok", [128, 2, S], F32, kind="ExternalOutput")
        di["dbg_kst"] = nc.dram_tensor("dbg_kst", [128, 2, S], SCORES_DT, kind="ExternalOutput")
        di["dbg_gate"] = nc.dram_tensor("dbg_gate", [MC, 6], F32, kind="ExternalOutput")
        di["dbg_qst"] = nc.dram_tensor("dbg_qst", [128, 2, NQ], SCORES_DT, kind="ExternalOutput")

    with tile.TileContext(nc) as tc:
        _build_tile(nc, tc, di, y)
    nc.compile()
    return nc


def _build_tile(nc, tc, di, y):
    with tc.tile_pool(name="cons", bufs=1) as cons:
        # ---- persistent small tiles ----
        ones_sb = cons.tile([128, 1], F32)
        nc.vector.memset(ones_sb[:], 1.0)
        eps1 = cons.tile([1, 1], F32)
        nc.vector.memset(eps1[:], EPS)
        epsc = cons.tile([128, 1], F32)
        nc.vector.memset(epsc[:], EPS)
        epsm = cons.tile([MC, 1], F32)
        nc.vector.memset(epsm[:], EPS)
        ones_r = cons.tile([128, 1], PROJ_DT)
        nc.vector.tensor_copy(ones_r[:], ones_sb[:])
        cv = cons.tile([128, 2, 8], F32)
        pe_sb = cons.tile([128, 2, W], F32)
        qw_sb = cons.tile([128, 2, C], PROJ_DT)
        kw_sb = cons.tile([128, 2, C], PROJ_DT)
        vw_sb = cons.tile([128, 2, C], PROJ_DT)
        ow_sb = cons.tile([128, 2, C], OPROJ_DT)
        rv_sb = cons.tile([128, 2, C], F32)
        vsn_sb = cons.tile([1, C], PROJ_DT)
        sr_sb = cons.tile([1, 2, 2, 128], PROJ_DT)
        gw_sb = cons.tile([128, 2, 1], F32)
        gv_sb = cons.tile([MC, 2], F32)
        sel_sb = cons.tile([128, 2, NQ], F32)
        posrow = cons.tile([128, 2, 3, W], F32)   # (cc, rowtype, w)
        dtop = cons.tile([128, 2, W], F32)
        dbot = cons.tile([128, 2, W], F32)
        # big persistent state
        tok = cons.tile([128, 2, S], F32)
        tok_r = cons.tile([128, 2, S], PROJ_DT)
        tokq = cons.tile([128, 2, NQ], F32)
        tokq_r = cons.tile([128, 2, NQ], PROJ_DT)
        kst = cons.tile([128, 2, S], SCORES_DT)
        qst = cons.tile([128, 2, NQ], SCORES_DT)
        v_tok = cons.tile([128, 18, NH, 33], AV_DT)
        murow = cons.tile([1, S], PROJ_DT)          # channel-SUM row (256*mu)
        rssc = cons.tile([128, 18], F32)        # rs(key) columns
        qmurow = cons.tile([1, NQ], PROJ_DT)    # q mean row
        qrsrow = cons.tile([1, NQ], F32)        # q rs row
        rsqbc = cons.tile([128, NQ], F32)       # SCALE * rs_q broadcast
        gate_sb = cons.tile([MC, 6], F32)
        xqres_sb = cons.tile([MC, 6, C], F32)

        with tc.tile_pool(name="work", bufs=2) as wk, \
             tc.tile_pool(name="worksm", bufs=3) as wks, \
             tc.tile_pool(name="pro", bufs=1) as pro, \
             tc.tile_pool(name="att2", bufs=2) as att2, \
             tc.tile_pool(name="atte", bufs=3) as atte, \
             tc.tile_pool(name="avn", bufs=2) as avnp, \
             tc.tile_pool(name="ps_s", bufs=2, space="PSUM") as ps_s, \
             tc.tile_pool(name="ps_av", bufs=1, space="PSUM") as ps_av, \
             tc.tile_pool(name="ps_m", bufs=2, space="PSUM") as ps_m:

            # ---- kick off the big input DMAs first ----
            xk_re = di["xk"].rearrange("(c p) s -> p c s", p=128)
            for m in range(NCH):
                nc.sync.dma_start(out=tok[:, :, m * MEGA:(m + 1) * MEGA],
                                  in_=xk_re[:, :, m * MEGA:(m + 1) * MEGA])
            nc.sync.dma_start(out=tokq,
                              in_=di["xq"].rearrange("(c p) s -> p c s", p=128))
            nc.sync.dma_start(out=cv, in_=di["cvecs"].rearrange("(c p) v -> p c v", p=128))
            nc.sync.dma_start(out=pe_sb, in_=di["pe"].rearrange("(c p) w -> p c w", p=128))

            # ================= text modulation MLP (c-major) =================
            text_sb = pro.tile([128, 4, 1], F32, tag="text")
            nc.sync.dma_start(out=text_sb,
                              in_=di["text"].rearrange("(k p) o -> p k o", p=128))
            w1_sb = pro.tile([128, 4, C], F32, tag="w1")
            nc.sync.dma_start(out=w1_sb,
                              in_=di["tmw1"].rearrange("(k p) d -> p k d", p=128))
            w2_sb = pro.tile([128, 2, C], F32, tag="w2")
            nc.sync.dma_start(out=w2_sb,
                              in_=di["tmw2"].rearrange("(k p) d -> p k d", p=128))

            def cmajor_mlp_layer(x_col, w_sb, nkc, bias_col, tag):
                h_col = wks.tile([128, 2, 1], F32, tag=f"{tag}_h")
                for c2c in range(2):
                    h_ps = ps_m.tile([128, 1], F32, tag="ps")
                    for kc in range(nkc):
                        nc.tensor.matmul(
                            h_ps[:, :], w_sb[:, kc, c2c * 128:(c2c + 1) * 128],
                            x_col[:, kc, :], start=(kc == 0), stop=(kc == nkc - 1))
                    nc.scalar.activation(h_col[:, c2c, :], h_ps[:, :], AF.Identity,
                                         bias=bias_col[:, c2c, :])
                return h_col

            def cmajor_ln_stats(h_col, tag):
                sum_ps = ps_m.tile([1, 2], F32, tag="ps")
                hsq = wks.tile([128, 2, 1], F32, tag=f"{tag}_hsq")
                nc.vector.tensor_mul(hsq[:], h_col[:], h_col[:])
                for cc in range(2):
                    nc.tensor.matmul(sum_ps[:, 0:1], ones_sb[:], h_col[:, cc, :],
                                     start=(cc == 0), stop=(cc == 1))
                    nc.tensor.matmul(sum_ps[:, 1:2], ones_sb[:], hsq[:, cc, :],
                                     start=(cc == 0), stop=(cc == 1))
                ms = wks.tile([1, 2], F32, tag=f"{tag}_ms")
                nc.vector.tensor_scalar_mul(ms[:], sum_ps[:, :], 1.0 / 256.0)
                var1 = wks.tile([1, 1], F32, tag=f"{tag}_var1")
                nc.vector.tensor_mul(var1[:], ms[:, 0:1], ms[:, 0:1])
                nc.vector.tensor_sub(var1[:], ms[:, 1:2], var1[:])
                nc.scalar.activation(var1[:], var1[:], AF.Ln, bias=eps1[:], scale=1.0)
                nc.scalar.activation(var1[:], var1[:], AF.Exp, scale=-0.5)
                mu_b = wks.tile([128, 1], F32, tag=f"{tag}_mub")
                nc.gpsimd.partition_broadcast(mu_b[:], ms[:, 0:1])
                rs_b = wks.tile([128, 1], F32, tag=f"{tag}_rsb")
                nc.gpsimd.partition_broadcast(rs_b[:], var1[:])
                return mu_b, rs_b

            h1 = cmajor_mlp_layer(text_sb, w1_sb, 4, cv[:, :, CV_TMB1:CV_TMB1 + 1], "l1")
            mu_b, rs_b = cmajor_ln_stats(h1, "l1")
            h1n = wks.tile([128, 2, 1], F32, tag="h1n")
            for cc in range(2):
                nc.vector.scalar_tensor_tensor(
                    h1n[:, cc, :], h1[:, cc, :], mu_b[:, 0:1], rs_b[:],
                    OP.subtract, OP.mult)
                nc.scalar.activation(h1n[:, cc, :], h1n[:, cc, :], AF.Relu,
                                     bias=cv[:, cc, CV_L1B:CV_L1B + 1],
                                     scale=cv[:, cc, CV_L1G:CV_L1G + 1])
            h2 = cmajor_mlp_layer(h1n, w2_sb, 2, cv[:, :, CV_TMB2:CV_TMB2 + 1], "l2")
            mu2_b, rs2_b = cmajor_ln_stats(h2, "l2")
            mod = wks.tile([128, 2, 1], F32, tag="mod")
            for cc in range(2):
                nc.vector.scalar_tensor_tensor(
                    mod[:, cc, :], h2[:, cc, :], mu2_b[:, 0:1], rs2_b[:],
                    OP.subtract, OP.mult)
                # sigmoid(z) = 1/(1+exp(-z)) via pre-negated g, b
                nc.scalar.activation(mod[:, cc, :], mod[:, cc, :], AF.Exp,
                                     bias=cv[:, cc, CV_L2BN:CV_L2BN + 1],
                                     scale=cv[:, cc, CV_L2GN:CV_L2GN + 1])
                nc.vector.tensor_scalar(mod[:, cc, :], mod[:, cc, :], 1.0, None, OP.add)
                nc.vector.reciprocal(mod[:, cc, :], mod[:, cc, :])

            # ---- conditional positional rows: 3 distinct conv rows (bf16) ----
            w3_sb = pro.tile([128, 3, 6, C], F32, tag="w3")
            nc.sync.dma_start(out=w3_sb,
                              in_=di["w3b"].rearrange("t (j p) m -> p t j m", p=128))
            # non-critical loads queued behind the prologue-critical DMAs
            nc.sync.dma_start(out=qw_sb, in_=di["qwg"].rearrange("(c p) d -> p c d", p=128))
            nc.sync.dma_start(out=kw_sb, in_=di["kwg"].rearrange("(c p) d -> p c d", p=128))
            nc.sync.dma_start(out=vw_sb, in_=di["vwg"].rearrange("(c p) d -> p c d", p=128))
            nc.sync.dma_start(out=ow_sb, in_=di["owg"][:, :, :])
            nc.sync.dma_start(out=rv_sb, in_=di["rowvecs"][:, :, :])
            nc.sync.dma_start(out=vsn_sb, in_=di["vsn"][:, :])
            nc.sync.dma_start(out=sr_sb, in_=di["sumrows"][:, :, :, :])
            nc.sync.dma_start(out=gw_sb, in_=di["gwg"].rearrange("(c p) o -> p c o", p=128))
            nc.sync.dma_start(out=gv_sb, in_=di["gvec"][:, :])
            nc.sync.dma_start(out=sel_sb, in_=di["selmask"][:, :, :])
            nc.sync.dma_start(out=xqres_sb,
                              in_=di["xqres"].rearrange("(k p) c -> p k c", p=MC))

            inrow = wks.tile([128, 2, W], F32, tag="inrow")
            for cc in range(2):
                nc.vector.tensor_scalar_mul(inrow[:, cc, :], pe_sb[:, cc, :],
                                            mod[:, cc, 0:1])
            im2 = wks.tile([128, 6, W], F32, tag="im2")
            nc.vector.memset(im2[:], 0.0)
            for kw in range(3):
                for cc in range(2):
                    j = kw * 2 + cc
                    if kw == 0:
                        nc.vector.tensor_copy(im2[:, j, 1:W], inrow[:, cc, 0:W - 1])
                    elif kw == 1:
                        nc.vector.tensor_copy(im2[:, j, :], inrow[:, cc, :])
                    else:
                        nc.vector.tensor_copy(im2[:, j, 0:W - 1], inrow[:, cc, 1:W])
            cps = ps_m.tile([128, 3, 2, W], F32, tag="ps")
            for t in range(3):
                for oc in range(2):
                    for j in range(6):
                        nc.tensor.matmul(cps[:, t, oc, :],
                                         w3_sb[:, t, j, oc * 128:(oc + 1) * 128],
                                         im2[:, j, :],
                                         start=(j == 0), stop=(j == 5))
            for cc in range(2):
                nc.vector.tensor_scalar(posrow[:, cc, :, :], cps[:, :, cc, :],
                                        cv[:, cc, CV_CONVB:CV_CONVB + 1], None, OP.add)
                nc.vector.tensor_sub(dtop[:, cc, :], posrow[:, cc, 0, :],
                                     posrow[:, cc, 1, :])
                nc.vector.tensor_sub(dbot[:, cc, :], posrow[:, cc, 2, :],
                                     posrow[:, cc, 1, :])

            # ================= q-side: tokens, stats, projections ============
            seltmp = pro.tile([128, NQ], F32, tag="seltmp")
            for cc in range(2):
                mid = posrow[:, cc, 1:2, :].to_broadcast([128, NQ // W, W])
                tq2 = tokq[:, cc, :].rearrange("p (h w) -> p h w", w=W)
                nc.vector.tensor_tensor(tq2, tq2, mid, OP.add)
                nc.vector.tensor_tensor(
                    seltmp[:].rearrange("p (h w) -> p h w", w=W),
                    sel_sb[:, 0, :].rearrange("p (h w) -> p h w", w=W),
                    dtop[:, cc, None, :].to_broadcast([128, NQ // W, W]), OP.mult)
                nc.vector.tensor_add(tokq[:, cc, :], tokq[:, cc, :], seltmp[:])
                nc.vector.tensor_tensor(
                    seltmp[:].rearrange("p (h w) -> p h w", w=W),
                    sel_sb[:, 1, :].rearrange("p (h w) -> p h w", w=W),
                    dbot[:, cc, None, :].to_broadcast([128, NQ // W, W]), OP.mult)
                nc.vector.tensor_add(tokq[:, cc, :], tokq[:, cc, :], seltmp[:])

            nc.sync.dma_start(out=tokq_r, in_=tokq.bitcast(PROJ_DT)[:])

            # q stats in column form ([96, 6] chunks)
            sqq = pro.tile([128, 2, NQ], F32, tag="sqq")
            nc.vector.tensor_mul(sqq[:], tokq[:], tokq[:])
            scolq = ps_m.tile([MC, 6, 2], F32, tag="ps")
            for ch in range(6):
                for cc in range(2):
                    nc.tensor.matmul(scolq[:, ch, 0:1],
                                     tokq[:, cc, ch * MC:(ch + 1) * MC],
                                     ones_sb[:], start=(cc == 0), stop=(cc == 1))
                    nc.tensor.matmul(scolq[:, ch, 1:2],
                                     sqq[:, cc, ch * MC:(ch + 1) * MC],
                                     ones_sb[:], start=(cc == 0), stop=(cc == 1))
            mrq = wks.tile([MC, 6, 2], F32, tag="mrq")   # [:,:,0]=mu  [:,:,1]=rs
            nc.vector.tensor_scalar_mul(mrq[:], scolq[:], 1.0 / 256.0)
            varq = wks.tile([MC, 6], F32, tag="varq")
            nc.vector.tensor_mul(varq[:], mrq[:, :, 0], mrq[:, :, 0])
            nc.vector.tensor_sub(varq[:], mrq[:, :, 1], varq[:])
            nc.scalar.activation(mrq[:, :, 1], varq[:], AF.Ln, bias=epsm[:], scale=1.0)
            nc.scalar.activation(mrq[:, :, 1], mrq[:, :, 1], AF.Exp, scale=-0.5)
            # q-side row sums (mu row for the q-proj correction; rs_q row)
            sqrow = pro.tile([1, NQ], F32, tag="sqrow")
            for half in range(2):
                h0 = half * IT
                mrow_ps = ps_m.tile([1, IT], F32, tag="ps")
                for cc in range(2):
                    nc.tensor.matmul(mrow_ps[:, :], ones_r[:],
                                     tokq_r[:, cc, h0:h0 + IT],
                                     start=(cc == 0), stop=(cc == 1))
                nc.vector.tensor_copy(qmurow[:, h0:h0 + IT], mrow_ps[:, :])
                srow_ps = ps_m.tile([1, IT], F32, tag="ps")
                for cc in range(2):
                    nc.tensor.matmul(srow_ps[:, :], ones_sb[:],
                                     sqq[:, cc, h0:h0 + IT],
                                     start=(cc == 0), stop=(cc == 1))
                nc.vector.tensor_copy(sqrow[:, h0:h0 + IT], srow_ps[:, :])
            # rs_q row: rsqrt(sumsq/256 - (sum/256)^2 + eps)
            qsc = pro.tile([1, NQ], F32, tag="qsc")
            nc.vector.tensor_scalar_mul(qsc[:], qmurow.bitcast(F32)[:], 1.0 / 256.0)
            nc.vector.tensor_mul(qsc[:], qsc[:], qsc[:])
            nc.vector.scalar_tensor_tensor(qsc[:], sqrow[:], 1.0 / 256.0, qsc[:],
                                           OP.mult, OP.subtract)
            nc.scalar.activation(qsc[:], qsc[:], AF.Ln, bias=eps1[:], scale=1.0)
            nc.scalar.activation(qsc[:], qsc[:], AF.Exp, scale=-0.5)
            nc.vector.tensor_scalar_mul(qsc[:], qsc[:], SCALE)
            nc.gpsimd.partition_broadcast(rsqbc[:], qsc[:])

            # q projection (fused LN): psum = qwg^T tokq - mu_q x qwg_sum
            for dc in range(2):
                for half in range(2):
                    h0 = half * IT
                    qp = ps_m.tile([128, IT], F32, tag="ps")
                    for cc in range(2):
                        nc.tensor.matmul(qp[:, :],
                                         qw_sb[:, cc, dc * 128:(dc + 1) * 128],
                                         tokq_r[:, cc, h0:h0 + IT],
                                         start=(cc == 0), stop=False)
                    nc.tensor.matmul(qp[:, :], sr_sb[0:1, 1, dc, :],
                                     qmurow[0:1, h0:h0 + IT],
                                     start=False, stop=True)
                    # qst = SCALE*rs_q (.) psum + qb_total x 1
                    tq = att2.tile([128, IT], F32, tag="tq")
                    nc.vector.tensor_mul(tq[:], qp[:, :], rsqbc[:, h0:h0 + IT])
                    nc.vector.tensor_scalar(
                        qst[:, dc, h0:h0 + IT], tq[:],
                        cv[:, dc, CV_QBT:CV_QBT + 1], None, OP.add)

            # gate logits (fused LN), column form [96, 6]
            gcol = ps_m.tile([MC, 6], F32, tag="ps")
            for ch in range(6):
                for cc in range(2):
                    nc.tensor.matmul(gcol[:, ch:ch + 1],
                                     tokq[:, cc, ch * MC:(ch + 1) * MC],
                                     gw_sb[:, cc, :], start=(cc == 0), stop=(cc == 1))
            glog = wks.tile([MC, 6], F32, tag="glog")
            # glog = rs_q .* (raw - mu_q * gwg_sum) + gb_total
            nc.vector.tensor_scalar_mul(glog[:], mrq[:, :, 0], gv_sb[:, 0:1])
            nc.vector.tensor_sub(glog[:], gcol[:, :], glog[:])
            nc.vector.tensor_mul(glog[:], glog[:], mrq[:, :, 1])
            nc.vector.tensor_scalar(glog[:], glog[:], gv_sb[:, 1:2], None, OP.add)
            eg = wks.tile([MC, 6], F32, tag="eg")
            nc.scalar.activation(eg[:], glog[:], AF.Exp, scale=-1.0)
            nc.vector.tensor_scalar(gate_sb[:], eg[:], 1.0, None, OP.add)
            nc.vector.reciprocal(gate_sb[:], gate_sb[:])

            # ================= k-side mega-chunk pipeline ====================
            def kside_mega(m):
                t0 = m * MEGA
                nrows = MEGA // W  # 24
                for cc in range(2):
                    r0 = 0
                    if m == 0:   # image top row
                        nc.vector.tensor_add(tok[:, cc, 0:W], tok[:, cc, 0:W],
                                             posrow[:, cc, 0, :])
                        r0 = 1
                    r1 = nrows
                    if m == NCH - 1:  # image bottom row
                        nc.vector.tensor_add(tok[:, cc, t0 + MEGA - W:t0 + MEGA],
                                             tok[:, cc, t0 + MEGA - W:t0 + MEGA],
                                             posrow[:, cc, 2, :])
                        r1 = nrows - 1
                    a, b = t0 + r0 * W, t0 + r1 * W
                    mid = posrow[:, cc, 1:2, :].to_broadcast([128, r1 - r0, W])
                    tv = tok[:, cc, a:b].rearrange("p (h w) -> p h w", w=W)
                    nc.vector.tensor_tensor(tv, tv, mid, OP.add)
                # stats columns
                sq = pro.tile([128, 2, MEGA], F32, tag="sqk")
                nc.vector.tensor_mul(sq[:], tok[:, :, t0:t0 + MEGA],
                                     tok[:, :, t0:t0 + MEGA])
                nc.sync.dma_start(out=tok_r[:, :, t0:t0 + MEGA],
                                  in_=tok.bitcast(PROJ_DT)[:, :, t0:t0 + MEGA])
                scol = ps_m.tile([128, NJM, 2], F32, tag="ps")
                for ch in range(NJM):
                    a = t0 + ch * 128
                    for cc in range(2):
                        nc.tensor.matmul(scol[:, ch, 0:1], tok[:, cc, a:a + 128],
                                         ones_sb[:], start=(cc == 0), stop=(cc == 1))
                        nc.tensor.matmul(scol[:, ch, 1:2],
                                         sq[:, cc, ch * 128:(ch + 1) * 128],
                                         ones_sb[:], start=(cc == 0), stop=(cc == 1))
                mm = wks.tile([128, NJM, 2], F32, tag="mmk")
                nc.vector.tensor_scalar_mul(mm[:], scol[:], 1.0 / 256.0)
                var = wks.tile([128, NJM], F32, tag="vark")
                nc.vector.tensor_mul(var[:], mm[:, :, 0], mm[:, :, 0])
                nc.vector.tensor_sub(var[:], mm[:, :, 1], var[:])
                rsk = rssc[:, m * NJM:(m + 1) * NJM]
                nc.scalar.activation(rsk, var[:], AF.Ln, bias=epsc[:], scale=1.0)
                nc.scalar.activation(rsk, rsk, AF.Exp, scale=-0.5)
                # channel-SUM rows (mu-correction moving operand)
                for quar in range(4):
                    a = t0 + quar * IT
                    mrow_ps = ps_m.tile([1, IT], F32, tag="ps")
                    for cc in range(2):
                        nc.tensor.matmul(mrow_ps[:, :], ones_r[:],
                                         tok_r[:, cc, a:a + IT],
                                         start=(cc == 0), stop=(cc == 1))
                    nc.vector.tensor_copy(murow[:, a:a + IT], mrow_ps[:, :])
                # k projection (fused LN, no bias; rs rides in the exp scale)
                for dc in range(2):
                    for quar in range(4):
                        a = t0 + quar * IT
                        kp = ps_m.tile([128, IT], F32, tag="ps")
                        for cc in range(2):
                            nc.tensor.matmul(kp[:, :],
                                             kw_sb[:, cc, dc * 128:(dc + 1) * 128],
                                             tok_r[:, cc, a:a + IT],
                                             start=(cc == 0), stop=False)
                        nc.tensor.matmul(kp[:, :], sr_sb[0:1, 0, dc, :],
                                         murow[0:1, a:a + IT],
                                         start=False, stop=True)
                        nc.vector.tensor_copy(kst[:, dc, a:a + IT], kp[:, :])
                # v projection (fused LN; rs applied on the psum->sbuf copy)
                for ch in range(NJM):
                    jc = m * NJM + ch
                    a = t0 + ch * 128
                    vp = ps_m.tile([128, C], F32, tag="ps")
                    for cc in range(2):
                        nc.tensor.matmul(vp[:, :],
                                         tok_r[:, cc, a:a + 128],
                                         vw_sb[:, cc, :], start=(cc == 0), stop=False)
                    nc.tensor.matmul(vp[:, :], murow[0:1, a:a + 128],
                                     vsn_sb[0:1, :], start=False, stop=True)
                    nc.vector.tensor_scalar(
                        v_tok[:, jc, :, 0:32], vp[:, :].rearrange(
                            "p (h d) -> p h d", d=32),
                        rssc[:, jc:jc + 1], None, OP.mult)
                nc.vector.tensor_copy(
                    v_tok[:, m * NJM:(m + 1) * NJM, :, 32:33],
                    ones_sb[:, None, None, :].to_broadcast([128, NJM, NH, 1]))

            kside_mega(0)

            if "dbg_tok" in di:
                kside_mega(1)
                nc.sync.dma_start(out=di["dbg_tok"][:, :, :], in_=tok[:])
                nc.sync.dma_start(out=di["dbg_kst"][:, :, :], in_=kst[:])
                nc.sync.dma_start(out=di["dbg_gate"][:, :], in_=gate_sb[:])
                nc.sync.dma_start(out=di["dbg_qst"][:, :, :], in_=qst[:])

            # ================= attention + epilogue ==========================
            state = {}

            def attn_block(it, p, m):
                i0 = it * IT
                av_ps = state["av_ps"]
                for ch in range(NJM):
                    jc = m * NJM + ch
                    s_ps = ps_s.tile([128, 2, 512], F32, tag="sps")
                    for hh in range(2):
                        h = 2 * p + hh
                        dc, poff = h // 4, 32 * (h % 4)
                        nc.tensor.matmul(
                            s_ps[:, hh, 0:IT],
                            kst[poff:poff + 32, dc, jc * 128:(jc + 1) * 128],
                            qst[poff:poff + 32, dc, i0:i0 + IT],
                            start=True, stop=True, tile_position=(poff, 0))
                    e_sb = atte.tile([128, 2, IT], AV_DT, tag="esb")
                    nc.scalar.activation(e_sb[:, :, :], s_ps[:, :, 0:IT],
                                         AF.Exp, scale=rssc[:, jc:jc + 1])
                    for hh in range(2):
                        nc.tensor.matmul(
                            av_ps[:, hh, 0:IT], v_tok[:, jc, 2 * p + hh, :],
                            e_sb[:, hh, :],
                            start=(jc == 0), stop=(jc == 17))

            for it in range(2):
                av_n = avnp.tile([128, 2, IT], OPROJ_DT, tag="avn")
                for p in range(4):
                    av_ps = ps_av.tile([33, 2, 512], F32, tag="avps")
                    state["av_ps"] = av_ps
                    attn_block(it, p, 0)
                    if it == 0 and p == 0 and "dbg_tok" not in di:
                        kside_mega(1)
                    attn_block(it, p, 1)
                    # normalize: av_n[h] = av[h] * (1 / l[h])
                    av_ps = state["av_ps"]
                    r_sb = att2.tile([1, 2, IT], F32, tag="rsb")
                    nc.vector.reciprocal(r_sb[:], av_ps[32:33, :, 0:IT])
                    r_bc = att2.tile([32, 2, IT], F32, tag="rbc")
                    nc.gpsimd.partition_broadcast(r_bc[:], r_sb[:])
                    for hh in range(2):
                        h = 2 * p + hh
                        g, poff = h // 4, 32 * (h % 4)
                        nc.vector.tensor_tensor(
                            av_n[poff:poff + 32, g, :],
                            av_ps[0:32, hh, 0:IT], r_bc[:, hh, :], OP.mult)
                # output projection + epilogue per 96-token chunk
                for mc in range(3):
                    ch = it * 3 + mc
                    o_ps = ps_m.tile([MC, C], F32, tag="ps")
                    for g in range(2):
                        nc.tensor.matmul(o_ps[:, :],
                                         av_n[:, g, mc * MC:(mc + 1) * MC],
                                         ow_sb[:, g, :], start=(g == 0), stop=(g == 1))
                    og = wk.tile([MC, C], F32, tag="og")
                    nc.vector.tensor_add(og[:], o_ps[:, :], rv_sb[0:MC, RV_OB, :])
                    nc.vector.tensor_scalar_mul(og[:], og[:], gate_sb[:, ch:ch + 1])
                    stats = wks.tile([MC, nc.vector.BN_STATS_DIM], F32, tag="bst")
                    nc.vector.bn_stats(stats[:], og[:])
                    mv = wks.tile([MC, 2], F32, tag="bag")
                    nc.vector.bn_aggr(mv[:], stats[:])
                    rs2 = wks.tile([MC, 1], F32, tag="eprs")
                    nc.scalar.activation(rs2[:], mv[:, 1:2], AF.Ln,
                                         bias=epsm[:], scale=1.0)
                    nc.scalar.activation(rs2[:], rs2[:], AF.Exp, scale=-0.5)
                    rsn = wks.tile([MC, C], F32, tag="rsn")
                    nc.vector.tensor_scalar_mul(rsn[:], rv_sb[0:MC, RV_NOG, :], rs2[:])
                    t2 = wk.tile([MC, C], F32, tag="ept2")
                    nc.vector.scalar_tensor_tensor(
                        t2[:], og[:], mv[:, 0:1], rsn[:], OP.subtract, OP.mult)
                    nc.vector.tensor_add(t2[:], t2[:], xqres_sb[:, ch, :])
                    nc.sync.dma_start(
                        out=y.rearrange("(k p) c -> p k c", p=MC)[:, ch, :], in_=t2[:])


def _host_inputs(x, text_feature, tm_w1, tm_b1, tm_ln1_g, tm_ln1_b, tm_w2, tm_b2,
                 tm_ln2_g, tm_ln2_b, conv_w, conv_b, q_w, q_b, k_w, k_b, v_w, v_b,
                 o_w, o_b, gate_w, nq_g, nq_b, nkv_g, nkv_b, no_g, no_b):
    f = np.float32
    # pe table (depends only on (c, w); faithful to reference)
    div = np.exp(np.arange(C // 2, dtype=f) * (-math.log(10000.0) / (C // 2)))
    wpos = np.arange(W, dtype=f)
    s = np.sin(wpos[None, :] * div[:, None])
    c = np.cos(wpos[None, :] * div[:, None])
    pe = np.stack([s, c], axis=1).reshape(C, W).astype(f)
    # kh-collapsed conv kernels: top(kh 1,2), mid(all), bot(kh 0,1)
    w3 = np.stack([
        conv_w[:, :, 1, :] + conv_w[:, :, 2, :],
        conv_w.sum(axis=2),
        conv_w[:, :, 0, :] + conv_w[:, :, 1, :],
    ]).astype(f)                                  # [3, Cout, Cin, kw]
    w3 = w3.transpose(0, 3, 2, 1).reshape(3, 768, C)  # [(kw, cin), cout]
    w3b = np.ascontiguousarray(w3, dtype=f)

    # LN-fused projection weights
    qwg = np.ascontiguousarray(q_w.T * nq_g[:, None], dtype=f)   # [c, d]
    kwg = np.ascontiguousarray(k_w.T * nkv_g[:, None], dtype=f)
    vwg = np.ascontiguousarray(v_w.T * nkv_g[:, None], dtype=f)  # [c, o]
    qb_total = (q_b + q_w @ nq_b).astype(f)
    # correction rows pair with channel-SUM rows -> fold the 1/256 here
    sumrows = np.stack([-kwg.sum(axis=0) / 256.0, -qwg.sum(axis=0) / 256.0]) \
        .reshape(1, 2, 2, 128).astype(f)
    # v bias (incl LN beta) folds through softmax-normalized attention
    vb_total = (v_b + v_w @ nkv_b).astype(f)
    ob_eff = (o_b + vb_total @ o_w.T).astype(f)
    # gate
    gwg = np.ascontiguousarray((gate_w[0] * nq_g)[:, None], dtype=f)  # [C, 1]
    gvec = np.zeros((MC, 2), f)
    gvec[:, 0] = gwg.sum()
    gvec[:, 1] = gate_w[0] @ nq_b
    # head-grouped output projection: partition 32*(h%4)+d, group h//4
    owg = np.zeros((128, 2, C), f)
    for h in range(NH):
        owg[32 * (h % 4):32 * (h % 4) + 32, h // 4, :] = o_w[:, 32 * h:32 * h + 32].T
    import ml_dtypes
    owg = owg.astype(ml_dtypes.bfloat16)
    rowvecs = np.zeros((128, 2, C), f)
    rowvecs[:, RV_OB, :] = ob_eff[None, :]
    rowvecs[:, RV_NOG, :] = no_g[None, :]
    vsn = np.ascontiguousarray((-vwg.sum(axis=0) / 256.0)[None, :], dtype=f)
    cvecs = np.stack([
        tm_b1, tm_ln1_g, tm_ln1_b, tm_b2, -tm_ln2_g, -tm_ln2_b, conv_b, qb_total,
    ], axis=1).astype(f)                          # [256, 8]

    per_core = []
    for core in range(8):
        b, k = core // 4, core % 4
        xb = np.ascontiguousarray(x[b].reshape(C, S), dtype=f)
        xqc = np.ascontiguousarray(xb[:, NQ * k:NQ * (k + 1)])
        sel = np.zeros((128, 2, NQ), f)
        if k == 0:
            sel[:, 0, 0:W] = 1.0
        if k == 3:
            sel[:, 1, NQ - W:NQ] = 1.0
        per_core.append({
            "xk": xb,
            "xq": xqc,
            "xqres": np.ascontiguousarray(xqc.T + no_b[None, :]),
            "text": np.ascontiguousarray(text_feature[b][:, None], dtype=f),
            "tmw1": np.ascontiguousarray(tm_w1.T, dtype=f),
            "tmw2": np.ascontiguousarray(tm_w2.T, dtype=f),
            "cvecs": cvecs, "pe": pe, "w3b": w3b,
            "qwg": qwg, "kwg": kwg, "vwg": vwg, "owg": owg,
            "rowvecs": rowvecs, "sumrows": sumrows, "vsn": vsn,
            "gwg": gwg, "gvec": gvec, "selmask": sel,
        })
    return per_core


_NC_CACHE = {}


def get_nc():
    if "nc" not in _NC_CACHE:
        _NC_CACHE["nc"] = build_bass()
    return _NC_CACHE["nc"]


def kernel(**inputs):
    inputs = {k: np.asarray(v, dtype=np.float32) for k, v in inputs.items()}
    in_maps = _host_inputs(**inputs)
    nc = get_nc()
    res = run_bass_kernel_spmd(nc, in_maps, core_ids=list(range(8)))
    x = inputs["x"]
    out = np.empty((B, C, H, W), np.float32)
    for b in range(B):
        blocks = [res.results[4 * b + k]["y"] for k in range(4)]  # [NQ, C] each
        tok = np.concatenate(blocks, axis=0)                      # [S, C]
        out[b] = tok.T.reshape(C, H, W)
    return out


# revision 39
# speedup vs baseline: 1.1733x; 1.0103x over previous
"""Trainium2 Bass kernel for nn_EnhancedTextAttentionBlock (v2).

Self-contained: takes FULL inputs (as in reference.setup_inputs()), shards
across 8 NeuronCores internally, returns the FULL [2, 256, 48, 48] output.

Sharding: core c handles batch b = c // 4 and query-token block k = c % 4
(576 of the 2304 spatial tokens). K/V are computed for the full token set on
every core; a single SPMD program serves all 8 cores with no collectives.

Algebraic restructurings (exact, not approximations):
- pe depends only on (c, w): the 3x3 conv collapses to three 1-D convs (bf16).
- LayerNorms are FUSED into the projections: with kn = (tok - mu)*rs*g + b,
  kst = rs ⊙ (kwg^T tok - mu ⊗ kwg_sum) where kwg = kw*g.  The mu-correction
  rides as an extra K=1 matmul accumulation chunk; the per-token rs rides as
  the exp()'s per-partition scale (keys) or is folded into qst (queries).
  kn/qn are never materialized.
- The k-projection bias shifts every score of a query by a constant and
  cancels in softmax -> dropped.  v's LN beta and bias commute through the
  softmax-normalized attention -> folded into the output bias on host.
- Softmax denominators ride as a ones-column of v; av is normalized before
  a head-grouped output projection (4 heads packed into K=128).
- Softmax max-subtraction is skipped: |scores| < ~2 in fp32 exp range.
- Channel-wise token stats are computed as matmul COLUMNS (free-dim 1), so
  the whole stats scalar chain runs on [128, nchunk] tiles (~free).
"""
import math
import numpy as np

import concourse.bass as bass
import concourse.tile as tile
from concourse import bacc, mybir
from concourse.bass_utils import run_bass_kernel_spmd

import os as _os
F32 = mybir.dt.float32
BF16 = mybir.dt.bfloat16
R = mybir.dt.float32r
_PREC = _os.environ.get("KERNEL_PREC", "allr")
PROJ_DT = F32 if _PREC == "f32" else R      # q/k/v projections (moving = tok)
SCORES_DT = F32 if _PREC in ("f32", "sf32") else R
AV_DT = F32 if _PREC in ("f32", "af32") else BF16   # v / exp(scores) storage
OPROJ_DT = BF16 if _PREC == "obf" else F32
AF = mybir.ActivationFunctionType
OP = mybir.AluOpType

B, C, H, W, T = 2, 256, 48, 48, 512
NH, HD = 8, 32
S = H * W              # 2304 tokens
NQ = S // 4            # 576 q tokens per core
SCALE = HD ** -0.5
IT = 288               # q-tile (two per q block)
MC = 96                # epilogue chunk
MEGA = 1152            # k-side pipeline chunk (2 per S)
NCH = S // MEGA        # 2 mega chunks
NJM = MEGA // 128      # 9 key chunks per mega
EPS = 1e-5

# cvecs column indices (c-major [256, 1] vectors packed into one input)
CV_TMB1, CV_L1G, CV_L1B, CV_TMB2, CV_L2GN, CV_L2BN, CV_CONVB, CV_QBT = range(8)
# rowvecs (token-major prebroadcast [128, 256] rows)
RV_OB, RV_NOG = range(2)


def build_bass():
    nc = bacc.Bacc("TRN2", target_bir_lowering=False, debug=False,
                   enable_asserts=True, num_devices=8)
    di = {}

    def inp(name, shape, dt=F32):
        di[name] = nc.dram_tensor(name, shape, dt, kind="ExternalInput")
        return di[name]

    inp("xk", [C, S])
    inp("xq", [C, NQ])
    inp("xqres", [NQ, C])          # xq^T + no_b (host-folded)
    inp("text", [T, 1])
    inp("tmw1", [T, C])
    inp("tmw2", [C, C])
    inp("cvecs", [C, 8])
    inp("pe", [C, W])
    inp("w3b", [3, 768, C])
    inp("qwg", [C, C], PROJ_DT)    # [c, d] = q_w[d, c] * nq_g[c]
    inp("kwg", [C, C], PROJ_DT)
    inp("vwg", [C, C], PROJ_DT)    # [c, o] = v_w[o, c] * nkv_g[c]
    inp("owg", [128, 2, C], OPROJ_DT)
    inp("rowvecs", [128, 2, C])
    inp("vsn", [1, C], PROJ_DT)
    inp("sumrows", [1, 2, 2, 128], PROJ_DT)  # [0, r, dc, d]: r0 = -kwg_sum, r1 = -qwg_sum
    inp("gwg", [C, 1])
    inp("gvec", [MC, 2])           # bc cols: gwg_sum, gb_total
    inp("selmask", [128, 2, NQ])
    y = nc.dram_tensor("y", [NQ, C], F32, kind="ExternalOutput")
    if _os.environ.get("KERNEL_DBG") == "1":
        di["dbg_tok"] = nc.dram_tensor("dbg_tok", [128, 2, S], F32, kind="ExternalOutput")
        di["dbg_kst"] = nc.dram_tensor("dbg_kst", [128, 2, S], SCORES_DT, kind="ExternalOutput")
        di["dbg_gate"] = nc.dram_tensor("dbg_gate", [MC, 6], F32, kind="ExternalOutput")
        di["dbg_qst"] = nc.dram_tensor("dbg_qst", [128, 2, NQ], SCORES_DT, kind="ExternalOutput")
        di["dbg_pos"] = nc.dram_tensor("dbg_pos", [128, 2, 3, W], F32, kind="ExternalOutput")
        di["dbg_mod"] = nc.dram_tensor("dbg_mod", [128, 2], F32, kind="ExternalOutput")
        di["dbg_mlp"] = nc.dram_tensor("dbg_mlp", [128, 2, 3], F32, kind="ExternalOutput")

    with tile.TileContext(nc) as tc:
        _build_tile(nc, tc, di, y)
    nc.compile()
    return nc


MAGIC = 0x5f3759df
I32 = mybir.dt.int32


def _rsqrt_dve(nc, pool, out_ap, x_ap, shape, tag):
    """out = 1/sqrt(x) via bit-trick seed + 2 Newton steps (DVE only).

    x_ap must be an SBUF fp32 AP (read-only); out_ap may alias a target slice.
    """
    y = pool.tile(shape, F32, tag=f"{tag}_y")
    t = pool.tile(shape, F32, tag=f"{tag}_t")
    nc.vector.tensor_scalar(y.bitcast(I32)[:], x_ap.bitcast(I32), 1, None,
                            OP.logical_shift_right)
    nc.vector.tensor_scalar(y.bitcast(I32)[:], y.bitcast(I32)[:], -1, MAGIC,
                            OP.mult, OP.add)
    for _ in range(2):
        nc.vector.tensor_mul(t[:], y[:], y[:])
        nc.vector.tensor_tensor(t[:], t[:], x_ap, OP.mult)
        nc.vector.tensor_scalar(t[:], t[:], -0.5, 1.5, OP.mult, OP.add)
        nc.vector.tensor_tensor(y[:], y[:], t[:], OP.mult)
    nc.vector.tensor_copy(out_ap, y[:])


def _build_tile(nc, tc, di, y):
    with tc.tile_pool(name="cons", bufs=1) as cons:
        # ---- persistent small tiles ----
        ones_sb = cons.tile([128, 1], F32)
        nc.vector.memset(ones_sb[:], 1.0)
        eps1 = cons.tile([1, 1], F32)
        nc.vector.memset(eps1[:], EPS)
        epsc = cons.tile([128, 1], F32)
        nc.vector.memset(epsc[:], EPS)
        epsm = cons.tile([MC, 1], F32)
        nc.vector.memset(epsm[:], EPS)
        ones_r = cons.tile([128, 1], PROJ_DT)
        nc.vector.tensor_copy(ones_r[:], ones_sb[:])
        cv = cons.tile([128, 2, 8], F32)
        pe_sb = cons.tile([128, 2, W], F32)
        qw_sb = cons.tile([128, 2, C], PROJ_DT)
        kw_sb = cons.tile([128, 2, C], PROJ_DT)
        vw_sb = cons.tile([128, 2, C], PROJ_DT)
        ow_sb = cons.tile([128, 2, C], OPROJ_DT)
        rv_sb = cons.tile([128, 2, C], F32)
        vsn_sb = cons.tile([1, C], PROJ_DT)
        sr_sb = cons.tile([1, 2, 2, 128], PROJ_DT)
        gw_sb = cons.tile([128, 2, 1], F32)
        gv_sb = cons.tile([MC, 2], F32)
        sel_sb = cons.tile([128, 2, NQ], F32)
        posrow = cons.tile([128, 2, 3, W], F32)   # (cc, rowtype, w)
        dtop = cons.tile([128, 2, W], F32)
        dbot = cons.tile([128, 2, W], F32)
        # big persistent state
        tok = cons.tile([128, 2, S], F32)
        tok_r = cons.tile([128, 2, S], PROJ_DT)
        tokq = cons.tile([128, 2, NQ], F32)
        tokq_r = cons.tile([128, 2, NQ], PROJ_DT)
        kst = cons.tile([128, 2, S], SCORES_DT)
        qst = cons.tile([128, 2, NQ], SCORES_DT)
        v_tok = cons.tile([128, 18, NH, 33], AV_DT)
        murow = cons.tile([1, S], PROJ_DT)          # channel-SUM row (256*mu)
        rssc = cons.tile([128, 18], F32)        # rs(key) columns
        qmurow = cons.tile([1, NQ], PROJ_DT)    # q mean row
        qrsrow = cons.tile([1, NQ], F32)        # q rs row
        rsqbc = cons.tile([128, NQ], F32)       # SCALE * rs_q broadcast
        gate_sb = cons.tile([MC, 6], F32)
        xqres_sb = cons.tile([MC, 6, C], F32)

        with tc.tile_pool(name="work", bufs=2) as wk, \
             tc.tile_pool(name="worksm", bufs=3) as wks, \
             tc.tile_pool(name="pro", bufs=1) as pro, \
             tc.tile_pool(name="att2", bufs=2) as att2, \
             tc.tile_pool(name="atte", bufs=3) as atte, \
             tc.tile_pool(name="avn", bufs=2) as avnp, \
             tc.tile_pool(name="ps_s", bufs=2, space="PSUM") as ps_s, \
             tc.tile_pool(name="ps_av", bufs=1, space="PSUM") as ps_av, \
             tc.tile_pool(name="ps_m", bufs=2, space="PSUM") as ps_m:

            # ---- kick off the big input DMAs first ----
            xk_re = di["xk"].rearrange("(c p) s -> p c s", p=128)
            for m in range(NCH):
                nc.sync.dma_start(out=tok[:, :, m * MEGA:(m + 1) * MEGA],
                                  in_=xk_re[:, :, m * MEGA:(m + 1) * MEGA])
            nc.sync.dma_start(out=tokq,
                              in_=di["xq"].rearrange("(c p) s -> p c s", p=128))
            nc.sync.dma_start(out=cv, in_=di["cvecs"].rearrange("(c p) v -> p c v", p=128))
            nc.sync.dma_start(out=pe_sb, in_=di["pe"].rearrange("(c p) w -> p c w", p=128))

            # ================= text modulation MLP (c-major) =================
            text_sb = pro.tile([128, 4, 1], F32, tag="text")
            nc.sync.dma_start(out=text_sb,
                              in_=di["text"].rearrange("(k p) o -> p k o", p=128))
            w1_sb = pro.tile([128, 4, C], F32, tag="w1")
            nc.sync.dma_start(out=w1_sb,
                              in_=di["tmw1"].rearrange("(k p) d -> p k d", p=128))
            w2_sb = pro.tile([128, 2, C], F32, tag="w2")
            nc.sync.dma_start(out=w2_sb,
                              in_=di["tmw2"].rearrange("(k p) d -> p k d", p=128))

            def cmajor_mlp_layer(x_col, w_sb, nkc, bias_col, tag):
                h_col = wks.tile([128, 2, 1], F32, tag=f"{tag}_h")
                for c2c in range(2):
                    h_ps = ps_m.tile([128, 1], F32, tag="ps")
                    for kc in range(nkc):
                        nc.tensor.matmul(
                            h_ps[:, :], w_sb[:, kc, c2c * 128:(c2c + 1) * 128],
                            x_col[:, kc, :], start=(kc == 0), stop=(kc == nkc - 1))
                    nc.scalar.activation(h_col[:, c2c, :], h_ps[:, :], AF.Identity,
                                         bias=bias_col[:, c2c, :])
                return h_col

            def cmajor_ln_stats(h_col, tag):
                sum_ps = ps_m.tile([1, 2], F32, tag="ps")
                hsq = wks.tile([128, 2, 1], F32, tag=f"{tag}_hsq")
                nc.vector.tensor_mul(hsq[:], h_col[:], h_col[:])
                for st, src_col in ((0, h_col), (1, hsq)):
                    for cc in range(2):
                        nc.tensor.matmul(sum_ps[:, st:st + 1], ones_sb[:],
                                         src_col[:, cc, :],
                                         start=(cc == 0), stop=(cc == 1))
                ms = wks.tile([1, 2], F32, tag=f"{tag}_ms")
                nc.vector.tensor_scalar_mul(ms[:], sum_ps[:, :], 1.0 / 256.0)
                var1 = wks.tile([1, 1], F32, tag=f"{tag}_var1")
                nc.vector.tensor_mul(var1[:], ms[:, 0:1], ms[:, 0:1])
                nc.vector.scalar_tensor_tensor(var1[:], ms[:, 1:2], EPS, var1[:],
                                               OP.add, OP.subtract)
                _rsqrt_dve(nc, wks, var1[:], var1[:], [1, 1], f"{tag}_rs")
                mu_b = wks.tile([128, 1], F32, tag=f"{tag}_mub")
                nc.gpsimd.partition_broadcast(mu_b[:], ms[:, 0:1])
                rs_b = wks.tile([128, 1], F32, tag=f"{tag}_rsb")
                nc.gpsimd.partition_broadcast(rs_b[:], var1[:])
                return mu_b, rs_b

            h1 = cmajor_mlp_layer(text_sb, w1_sb, 4, cv[:, :, CV_TMB1:CV_TMB1 + 1], "l1")
            mu_b, rs_b = cmajor_ln_stats(h1, "l1")
            h1n = wks.tile([128, 2, 1], F32, tag="h1n")
            for cc in range(2):
                nc.vector.scalar_tensor_tensor(
                    h1n[:, cc, :], h1[:, cc, :], mu_b[:, 0:1], rs_b[:],
                    OP.subtract, OP.mult)
                nc.scalar.activation(h1n[:, cc, :], h1n[:, cc, :], AF.Relu,
                                     bias=cv[:, cc, CV_L1B:CV_L1B + 1],
                                     scale=cv[:, cc, CV_L1G:CV_L1G + 1])
            h2 = cmajor_mlp_layer(h1n, w2_sb, 2, cv[:, :, CV_TMB2:CV_TMB2 + 1], "l2")
            if "dbg_mlp" in di:
                dmlp = cons.tile([128, 2, 3], F32)
                nc.vector.tensor_copy(dmlp[:, :, 0:1], h1[:])
                nc.vector.tensor_copy(dmlp[:, :, 1:2], h1n[:])
                nc.vector.tensor_copy(dmlp[:, :, 2:3], h2[:])
                nc.sync.dma_start(out=di["dbg_mlp"][:, :, :], in_=dmlp[:])
            mu2_b, rs2_b = cmajor_ln_stats(h2, "l2")
            mod = wks.tile([128, 2, 1], F32, tag="mod")
            state_mod = mod
            for cc in range(2):
                nc.vector.scalar_tensor_tensor(
                    mod[:, cc, :], h2[:, cc, :], mu2_b[:, 0:1], rs2_b[:],
                    OP.subtract, OP.mult)
                # sigmoid(z) = 1/(1+exp(-z)) via pre-negated g, b
                nc.scalar.activation(mod[:, cc, :], mod[:, cc, :], AF.Exp,
                                     bias=cv[:, cc, CV_L2BN:CV_L2BN + 1],
                                     scale=cv[:, cc, CV_L2GN:CV_L2GN + 1])
                nc.vector.tensor_scalar(mod[:, cc, :], mod[:, cc, :], 1.0, None, OP.add)
                nc.vector.reciprocal(mod[:, cc, :], mod[:, cc, :])

            # ---- conditional positional rows: 3 distinct conv rows (bf16) ----
            w3_sb = pro.tile([128, 3, 6, C], F32, tag="w3")
            nc.sync.dma_start(out=w3_sb,
                              in_=di["w3b"].rearrange("t (j p) m -> p t j m", p=128))
            # non-critical loads queued behind the prologue-critical DMAs
            nc.sync.dma_start(out=qw_sb, in_=di["qwg"].rearrange("(c p) d -> p c d", p=128))
            nc.sync.dma_start(out=kw_sb, in_=di["kwg"].rearrange("(c p) d -> p c d", p=128))
            nc.sync.dma_start(out=vw_sb, in_=di["vwg"].rearrange("(c p) d -> p c d", p=128))
            nc.sync.dma_start(out=ow_sb, in_=di["owg"][:, :, :])
            nc.sync.dma_start(out=rv_sb, in_=di["rowvecs"][:, :, :])
            nc.sync.dma_start(out=vsn_sb, in_=di["vsn"][:, :])
            nc.sync.dma_start(out=sr_sb, in_=di["sumrows"][:, :, :, :])
            nc.sync.dma_start(out=gw_sb, in_=di["gwg"].rearrange("(c p) o -> p c o", p=128))
            nc.sync.dma_start(out=gv_sb, in_=di["gvec"][:, :])
            nc.sync.dma_start(out=sel_sb, in_=di["selmask"][:, :, :])
            nc.sync.dma_start(out=xqres_sb,
                              in_=di["xqres"].rearrange("(k p) c -> p k c", p=MC))

            inrow = wks.tile([128, 2, W], F32, tag="inrow")
            for cc in range(2):
                nc.vector.tensor_scalar_mul(inrow[:, cc, :], pe_sb[:, cc, :],
                                            mod[:, cc, 0:1])
            im2 = wks.tile([128, 6, W], F32, tag="im2")
            nc.vector.memset(im2[:], 0.0)
            for kw in range(3):
                for cc in range(2):
                    j = kw * 2 + cc
                    if kw == 0:
                        nc.vector.tensor_copy(im2[:, j, 1:W], inrow[:, cc, 0:W - 1])
                    elif kw == 1:
                        nc.vector.tensor_copy(im2[:, j, :], inrow[:, cc, :])
                    else:
                        nc.vector.tensor_copy(im2[:, j, 0:W - 1], inrow[:, cc, 1:W])
            cps = ps_m.tile([128, 3, 2, W], F32, tag="ps")
            for t in range(3):
                for oc in range(2):
                    for j in range(6):
                        nc.tensor.matmul(cps[:, t, oc, :],
                                         w3_sb[:, t, j, oc * 128:(oc + 1) * 128],
                                         im2[:, j, :],
                                         start=(j == 0), stop=(j == 5))
            for cc in range(2):
                nc.vector.tensor_scalar(posrow[:, cc, :, :], cps[:, :, cc, :],
                                        cv[:, cc, CV_CONVB:CV_CONVB + 1], None, OP.add)
                nc.vector.tensor_sub(dtop[:, cc, :], posrow[:, cc, 0, :],
                                     posrow[:, cc, 1, :])
                nc.vector.tensor_sub(dbot[:, cc, :], posrow[:, cc, 2, :],
                                     posrow[:, cc, 1, :])

            # ================= q-side: tokens, stats, projections ============
            seltmp = pro.tile([128, NQ], F32, tag="seltmp")
            for cc in range(2):
                mid = posrow[:, cc, 1:2, :].to_broadcast([128, NQ // W, W])
                tq2 = tokq[:, cc, :].rearrange("p (h w) -> p h w", w=W)
                nc.vector.tensor_tensor(tq2, tq2, mid, OP.add)
                nc.vector.tensor_tensor(
                    seltmp[:].rearrange("p (h w) -> p h w", w=W),
                    sel_sb[:, 0, :].rearrange("p (h w) -> p h w", w=W),
                    dtop[:, cc, None, :].to_broadcast([128, NQ // W, W]), OP.mult)
                nc.vector.tensor_add(tokq[:, cc, :], tokq[:, cc, :], seltmp[:])
                nc.vector.tensor_tensor(
                    seltmp[:].rearrange("p (h w) -> p h w", w=W),
                    sel_sb[:, 1, :].rearrange("p (h w) -> p h w", w=W),
                    dbot[:, cc, None, :].to_broadcast([128, NQ // W, W]), OP.mult)
                nc.vector.tensor_add(tokq[:, cc, :], tokq[:, cc, :], seltmp[:])

            nc.sync.dma_start(out=tokq_r, in_=tokq.bitcast(PROJ_DT)[:])

            # q stats in column form ([96, 6] chunks)
            sqq = pro.tile([128, 2, NQ], F32, tag="sqq")
            nc.vector.tensor_mul(sqq[:], tokq[:], tokq[:])
            scolq = ps_m.tile([MC, 6, 2], F32, tag="ps")
            for ch in range(6):
                for st, srct in ((0, tokq), (1, sqq)):
                    for cc in range(2):
                        nc.tensor.matmul(scolq[:, ch, st:st + 1],
                                         srct[:, cc, ch * MC:(ch + 1) * MC],
                                         ones_sb[:], start=(cc == 0), stop=(cc == 1))
            mrq = wks.tile([MC, 6, 2], F32, tag="mrq")   # [:,:,0]=mu  [:,:,1]=rs
            nc.vector.tensor_scalar_mul(mrq[:], scolq[:], 1.0 / 256.0)
            varq = wks.tile([MC, 6], F32, tag="varq")
            nc.vector.tensor_mul(varq[:], mrq[:, :, 0], mrq[:, :, 0])
            nc.vector.scalar_tensor_tensor(varq[:], mrq[:, :, 1], EPS, varq[:],
                                           OP.add, OP.subtract)
            _rsqrt_dve(nc, wks, mrq[:, :, 1], varq[:], [MC, 6], "rsq")
            # q-side row sums (mu row for the q-proj correction; rs_q row)
            sqrow = pro.tile([1, NQ], F32, tag="sqrow")
            for half in range(2):
                h0 = half * IT
                mrow_ps = ps_m.tile([1, IT], F32, tag="ps")
                for cc in range(2):
                    nc.tensor.matmul(mrow_ps[:, :], ones_r[:],
                                     tokq_r[:, cc, h0:h0 + IT],
                                     start=(cc == 0), stop=(cc == 1))
                nc.vector.tensor_copy(qmurow[:, h0:h0 + IT], mrow_ps[:, :])
                srow_ps = ps_m.tile([1, IT], F32, tag="ps")
                for cc in range(2):
                    nc.tensor.matmul(srow_ps[:, :], ones_sb[:],
                                     sqq[:, cc, h0:h0 + IT],
                                     start=(cc == 0), stop=(cc == 1))
                nc.vector.tensor_copy(sqrow[:, h0:h0 + IT], srow_ps[:, :])
            # rs_q row: rsqrt(sumsq/256 - (sum/256)^2 + eps)
            qsc = pro.tile([1, NQ], F32, tag="qsc")
            nc.vector.tensor_scalar_mul(qsc[:], qmurow.bitcast(F32)[:], 1.0 / 256.0)
            nc.vector.tensor_mul(qsc[:], qsc[:], qsc[:])
            nc.vector.scalar_tensor_tensor(qsc[:], sqrow[:], 1.0 / 256.0, qsc[:],
                                           OP.mult, OP.subtract)
            nc.vector.tensor_scalar(qsc[:], qsc[:], EPS, None, OP.add)
            _rsqrt_dve(nc, pro, qsc[:], qsc[:], [1, NQ], "qscr")
            nc.vector.tensor_scalar_mul(qsc[:], qsc[:], SCALE)
            nc.gpsimd.partition_broadcast(rsqbc[:], qsc[:])

            # q projection (fused LN): psum = qwg^T tokq - mu_q x qwg_sum
            for dc in range(2):
                for half in range(2):
                    h0 = half * IT
                    qp = ps_m.tile([128, IT], F32, tag="ps")
                    for cc in range(2):
                        nc.tensor.matmul(qp[:, :],
                                         qw_sb[:, cc, dc * 128:(dc + 1) * 128],
                                         tokq_r[:, cc, h0:h0 + IT],
                                         start=(cc == 0), stop=False)
                    nc.tensor.matmul(qp[:, :], sr_sb[0:1, 1, dc, :],
                                     qmurow[0:1, h0:h0 + IT],
                                     start=False, stop=True)
                    # qst = SCALE*rs_q (.) psum + qb_total x 1
                    tq = att2.tile([128, IT], F32, tag="tq")
                    nc.vector.tensor_mul(tq[:], qp[:, :], rsqbc[:, h0:h0 + IT])
                    nc.scalar.activation(qst[:, dc, h0:h0 + IT], tq[:], AF.Identity,
                                         bias=cv[:, dc, CV_QBT:CV_QBT + 1])

            # gate logits (fused LN), column form [96, 6]
            gcol = ps_m.tile([MC, 6], F32, tag="ps")
            for ch in range(6):
                for cc in range(2):
                    nc.tensor.matmul(gcol[:, ch:ch + 1],
                                     tokq[:, cc, ch * MC:(ch + 1) * MC],
                                     gw_sb[:, cc, :], start=(cc == 0), stop=(cc == 1))
            glog = wks.tile([MC, 6], F32, tag="glog")
            # glog = rs_q .* (raw - mu_q * gwg_sum) + gb_total
            nc.vector.tensor_scalar_mul(glog[:], mrq[:, :, 0], gv_sb[:, 0:1])
            nc.vector.tensor_sub(glog[:], gcol[:, :], glog[:])
            nc.vector.tensor_mul(glog[:], glog[:], mrq[:, :, 1])
            nc.vector.tensor_scalar(glog[:], glog[:], gv_sb[:, 1:2], None, OP.add)
            eg = wks.tile([MC, 6], F32, tag="eg")
            nc.scalar.activation(eg[:], glog[:], AF.Exp, scale=-1.0)
            nc.vector.tensor_scalar(gate_sb[:], eg[:], 1.0, None, OP.add)
            nc.vector.reciprocal(gate_sb[:], gate_sb[:])

            # ================= k-side mega-chunk pipeline ====================
            def kside_mega(m):
                t0 = m * MEGA
                nrows = MEGA // W  # 24
                for cc in range(2):
                    r0 = 0
                    if m == 0:   # image top row
                        nc.vector.tensor_add(tok[:, cc, 0:W], tok[:, cc, 0:W],
                                             posrow[:, cc, 0, :])
                        r0 = 1
                    r1 = nrows
                    if m == NCH - 1:  # image bottom row
                        nc.vector.tensor_add(tok[:, cc, t0 + MEGA - W:t0 + MEGA],
                                             tok[:, cc, t0 + MEGA - W:t0 + MEGA],
                                             posrow[:, cc, 2, :])
                        r1 = nrows - 1
                    a, b = t0 + r0 * W, t0 + r1 * W
                    mid = posrow[:, cc, 1:2, :].to_broadcast([128, r1 - r0, W])
                    tv = tok[:, cc, a:b].rearrange("p (h w) -> p h w", w=W)
                    nc.vector.tensor_tensor(tv, tv, mid, OP.add)
                # stats columns
                sq = pro.tile([128, 2, MEGA], F32, tag="sqk")
                nc.vector.tensor_mul(sq[:], tok[:, :, t0:t0 + MEGA],
                                     tok[:, :, t0:t0 + MEGA])
                nc.sync.dma_start(out=tok_r[:, :, t0:t0 + MEGA],
                                  in_=tok.bitcast(PROJ_DT)[:, :, t0:t0 + MEGA])
                scol = ps_m.tile([128, NJM, 2], F32, tag="ps")
                for ch in range(NJM):
                    a = t0 + ch * 128
                    for cc in range(2):
                        nc.tensor.matmul(scol[:, ch, 0:1], tok[:, cc, a:a + 128],
                                         ones_sb[:], start=(cc == 0), stop=(cc == 1))
                    for cc in range(2):
                        nc.tensor.matmul(scol[:, ch, 1:2],
                                         sq[:, cc, ch * 128:(ch + 1) * 128],
                                         ones_sb[:], start=(cc == 0), stop=(cc == 1))
                mm = wks.tile([128, NJM, 2], F32, tag="mmk")
                nc.vector.tensor_scalar_mul(mm[:], scol[:], 1.0 / 256.0)
                var = wks.tile([128, NJM], F32, tag="vark")
                nc.vector.tensor_mul(var[:], mm[:, :, 0], mm[:, :, 0])
                nc.vector.scalar_tensor_tensor(var[:], mm[:, :, 1], EPS, var[:],
                                               OP.add, OP.subtract)
                rsk = rssc[:, m * NJM:(m + 1) * NJM]
                _rsqrt_dve(nc, wks, rsk, var[:], [128, NJM], "rsk")
                # channel-SUM rows (mu-correction moving operand)
                for quar in range(4):
                    a = t0 + quar * IT
                    mrow_ps = ps_m.tile([1, IT], F32, tag="ps")
                    for cc in range(2):
                        nc.tensor.matmul(mrow_ps[:, :], ones_r[:],
                                         tok_r[:, cc, a:a + IT],
                                         start=(cc == 0), stop=(cc == 1))
                    nc.vector.tensor_copy(murow[:, a:a + IT], mrow_ps[:, :])
                # k projection (fused LN, no bias; rs rides in the exp scale)
                for dc in range(2):
                    for quar in range(4):
                        a = t0 + quar * IT
                        kp = ps_m.tile([128, IT], F32, tag="ps")
                        for cc in range(2):
                            nc.tensor.matmul(kp[:, :],
                                             kw_sb[:, cc, dc * 128:(dc + 1) * 128],
                                             tok_r[:, cc, a:a + IT],
                                             start=(cc == 0), stop=False)
                        nc.tensor.matmul(kp[:, :], sr_sb[0:1, 0, dc, :],
                                         murow[0:1, a:a + IT],
                                         start=False, stop=True)
                        nc.scalar.copy(kst[:, dc, a:a + IT], kp[:, :])
                # v projection (fused LN; rs applied on the psum->sbuf copy)
                for ch in range(NJM):
                    jc = m * NJM + ch
                    a = t0 + ch * 128
                    vp = ps_m.tile([128, C], F32, tag="ps")
                    for cc in range(2):
                        nc.tensor.matmul(vp[:, :],
                                         tok_r[:, cc, a:a + 128],
                                         vw_sb[:, cc, :], start=(cc == 0), stop=False)
                    nc.tensor.matmul(vp[:, :], murow[0:1, a:a + 128],
                                     vsn_sb[0:1, :], start=False, stop=True)
                    nc.vector.tensor_scalar(
                        v_tok[:, jc, :, 0:32], vp[:, :].rearrange(
                            "p (h d) -> p h d", d=32),
                        rssc[:, jc:jc + 1], None, OP.mult)
                nc.vector.tensor_copy(
                    v_tok[:, m * NJM:(m + 1) * NJM, :, 32:33],
                    ones_sb[:, None, None, :].to_broadcast([128, NJM, NH, 1]))

            kside_mega(0)

            if "dbg_tok" in di:
                kside_mega(1)
                nc.sync.dma_start(out=di["dbg_tok"][:, :, :], in_=tok[:])
                nc.sync.dma_start(out=di["dbg_kst"][:, :, :], in_=kst[:])
                nc.sync.dma_start(out=di["dbg_gate"][:, :], in_=gate_sb[:])
                nc.sync.dma_start(out=di["dbg_qst"][:, :, :], in_=qst[:])
                nc.sync.dma_start(out=di["dbg_pos"][:, :, :, :], in_=posrow[:])
                nc.sync.dma_start(out=di["dbg_mod"][:, :], in_=state_mod[:, :, 0])

            # ================= attention + epilogue ==========================
            state = {"mod": state_mod}

            def attn_block(it, p, m):
                i0 = it * IT
                av_ps = state["av_ps"]
                for ch in range(NJM):
                    jc = m * NJM + ch
                    s_ps = ps_s.tile([128, 2, 512], F32, tag="sps")
                    for hh in range(2):
                        h = 2 * p + hh
                        dc, poff = h // 4, 32 * (h % 4)
                        nc.tensor.matmul(
                            s_ps[:, hh, 0:IT],
                            kst[poff:poff + 32, dc, jc * 128:(jc + 1) * 128],
                            qst[poff:poff + 32, dc, i0:i0 + IT],
                            start=True, stop=True, tile_position=(poff, 0))
                    e_sb = atte.tile([128, 2, IT], AV_DT, tag="esb")
                    nc.scalar.activation(e_sb[:, :, :], s_ps[:, :, 0:IT],
                                         AF.Exp, scale=rssc[:, jc:jc + 1])
                    for hh in range(2):
                        nc.tensor.matmul(
                            av_ps[:, hh, 0:IT], v_tok[:, jc, 2 * p + hh, :],
                            e_sb[:, hh, :],
                            start=(jc == 0), stop=(jc == 17))

            for it in range(2):
                av_n = avnp.tile([128, 2, IT], OPROJ_DT, tag="avn")
                for p in range(4):
                    av_ps = ps_av.tile([33, 2, 512], F32, tag="avps")
                    state["av_ps"] = av_ps
                    attn_block(it, p, 0)
                    if it == 0 and p == 0 and "dbg_tok" not in di:
                        kside_mega(1)
                    attn_block(it, p, 1)
                    # normalize: av_n[h] = av[h] * (1 / l[h])
                    av_ps = state["av_ps"]
                    r_sb = att2.tile([1, 2, IT], F32, tag="rsb")
                    nc.vector.reciprocal(r_sb[:], av_ps[32:33, :, 0:IT])
                    r_bc = att2.tile([32, 2, IT], F32, tag="rbc")
                    nc.gpsimd.partition_broadcast(r_bc[:], r_sb[:])
                    for hh in range(2):
                        h = 2 * p + hh
                        g, poff = h // 4, 32 * (h % 4)
                        nc.vector.tensor_tensor(
                            av_n[poff:poff + 32, g, :],
                            av_ps[0:32, hh, 0:IT], r_bc[:, hh, :], OP.mult)
                # output projection + epilogue per 96-token chunk
                for mc in range(3):
                    ch = it * 3 + mc
                    o_ps = ps_m.tile([MC, C], F32, tag="ps")
                    for g in range(2):
                        nc.tensor.matmul(o_ps[:, :],
                                         av_n[:, g, mc * MC:(mc + 1) * MC],
                                         ow_sb[:, g, :], start=(g == 0), stop=(g == 1))
                    og = wk.tile([MC, C], F32, tag="og")
                    nc.vector.tensor_add(og[:], o_ps[:, :], rv_sb[0:MC, RV_OB, :])
                    nc.vector.tensor_scalar_mul(og[:], og[:], gate_sb[:, ch:ch + 1])
                    stats = wks.tile([MC, nc.vector.BN_STATS_DIM], F32, tag="bst")
                    nc.vector.bn_stats(stats[:], og[:])
                    mv = wks.tile([MC, 2], F32, tag="bag")
                    nc.vector.bn_aggr(mv[:], stats[:])
                    rs2 = wks.tile([MC, 1], F32, tag="eprs")
                    nc.vector.tensor_scalar(rs2[:], mv[:, 1:2], EPS, None, OP.add)
                    _rsqrt_dve(nc, wks, rs2[:], rs2[:], [MC, 1], "eprsn")
                    rsn = wks.tile([MC, C], F32, tag="rsn")
                    nc.vector.tensor_scalar_mul(rsn[:], rv_sb[0:MC, RV_NOG, :], rs2[:])
                    t2 = wk.tile([MC, C], F32, tag="ept2")
                    nc.vector.scalar_tensor_tensor(
                        t2[:], og[:], mv[:, 0:1], rsn[:], OP.subtract, OP.mult)
                    nc.vector.tensor_add(t2[:], t2[:], xqres_sb[:, ch, :])
                    nc.sync.dma_start(
                        out=y.rearrange("(k p) c -> p k c", p=MC)[:, ch, :], in_=t2[:])


def _host_inputs(x, text_feature, tm_w1, tm_b1, tm_ln1_g, tm_ln1_b, tm_w2, tm_b2,
                 tm_ln2_g, tm_ln2_b, conv_w, conv_b, q_w, q_b, k_w, k_b, v_w, v_b,
                 o_w, o_b, gate_w, nq_g, nq_b, nkv_g, nkv_b, no_g, no_b):
    f = np.float32
    # pe table (depends only on (c, w); faithful to reference)
    div = np.exp(np.arange(C // 2, dtype=f) * (-math.log(10000.0) / (C // 2)))
    wpos = np.arange(W, dtype=f)
    s = np.sin(wpos[None, :] * div[:, None])
    c = np.cos(wpos[None, :] * div[:, None])
    pe = np.stack([s, c], axis=1).reshape(C, W).astype(f)
    # kh-collapsed conv kernels: top(kh 1,2), mid(all), bot(kh 0,1)
    w3 = np.stack([
        conv_w[:, :, 1, :] + conv_w[:, :, 2, :],
        conv_w.sum(axis=2),
        conv_w[:, :, 0, :] + conv_w[:, :, 1, :],
    ]).astype(f)                                  # [3, Cout, Cin, kw]
    w3 = w3.transpose(0, 3, 2, 1).reshape(3, 768, C)  # [(kw, cin), cout]
    w3b = np.ascontiguousarray(w3, dtype=f)

    # LN-fused projection weights
    qwg = np.ascontiguousarray(q_w.T * nq_g[:, None], dtype=f)   # [c, d]
    kwg = np.ascontiguousarray(k_w.T * nkv_g[:, None], dtype=f)
    vwg = np.ascontiguousarray(v_w.T * nkv_g[:, None], dtype=f)  # [c, o]
    qb_total = (q_b + q_w @ nq_b).astype(f)
    # correction rows pair with channel-SUM rows -> fold the 1/256 here
    sumrows = np.stack([-kwg.sum(axis=0) / 256.0, -qwg.sum(axis=0) / 256.0]) \
        .reshape(1, 2, 2, 128).astype(f)
    # v bias (incl LN beta) folds through softmax-normalized attention
    vb_total = (v_b + v_w @ nkv_b).astype(f)
    ob_eff = (o_b + vb_total @ o_w.T).astype(f)
    # gate
    gwg = np.ascontiguousarray((gate_w[0] * nq_g)[:, None], dtype=f)  # [C, 1]
    gvec = np.zeros((MC, 2), f)
    gvec[:, 0] = gwg.sum()
    gvec[:, 1] = gate_w[0] @ nq_b
    # head-grouped output projection: partition 32*(h%4)+d, group h//4
    owg = np.zeros((128, 2, C), f)
    for h in range(NH):
        owg[32 * (h % 4):32 * (h % 4) + 32, h // 4, :] = o_w[:, 32 * h:32 * h + 32].T
    if OPROJ_DT == BF16:
        import ml_dtypes
        owg = owg.astype(ml_dtypes.bfloat16)
    rowvecs = np.zeros((128, 2, C), f)
    rowvecs[:, RV_OB, :] = ob_eff[None, :]
    rowvecs[:, RV_NOG, :] = no_g[None, :]
    vsn = np.ascontiguousarray((-vwg.sum(axis=0) / 256.0)[None, :], dtype=f)
    cvecs = np.stack([
        tm_b1, tm_ln1_g, tm_ln1_b, tm_b2, -tm_ln2_g, -tm_ln2_b, conv_b, qb_total,
    ], axis=1).astype(f)                          # [256, 8]

    per_core = []
    for core in range(8):
        b, k = core // 4, core % 4
        xb = np.ascontiguousarray(x[b].reshape(C, S), dtype=f)
        xqc = np.ascontiguousarray(xb[:, NQ * k:NQ * (k + 1)])
        sel = np.zeros((128, 2, NQ), f)
        if k == 0:
            sel[:, 0, 0:W] = 1.0
        if k == 3:
            sel[:, 1, NQ - W:NQ] = 1.0
        per_core.append({
            "xk": xb,
            "xq": xqc,
            "xqres": np.ascontiguousarray(xqc.T + no_b[None, :]),
            "text": np.ascontiguousarray(text_feature[b][:, None], dtype=f),
            "tmw1": np.ascontiguousarray(tm_w1.T, dtype=f),
            "tmw2": np.ascontiguousarray(tm_w2.T, dtype=f),
            "cvecs": cvecs, "pe": pe, "w3b": w3b,
            "qwg": qwg, "kwg": kwg, "vwg": vwg, "owg": owg,
            "rowvecs": rowvecs, "sumrows": sumrows, "vsn": vsn,
            "gwg": gwg, "gvec": gvec, "selmask": sel,
        })
    return per_core


_NC_CACHE = {}


def get_nc():
    if "nc" not in _NC_CACHE:
        _NC_CACHE["nc"] = build_bass()
    return _NC_CACHE["nc"]


def kernel(**inputs):
    inputs = {k: np.asarray(v, dtype=np.float32) for k, v in inputs.items()}
    in_maps = _host_inputs(**inputs)
    nc = get_nc()
    res = run_bass_kernel_spmd(nc, in_maps, core_ids=list(range(8)))
    x = inputs["x"]
    out = np.empty((B, C, H, W), np.float32)
    for b in range(B):
        blocks = [res.results[4 * b + k]["y"] for k in range(4)]  # [NQ, C] each
        tok = np.concatenate(blocks, axis=0)                      # [S, C]
        out[b] = tok.T.reshape(C, H, W)
    return out


# revision 41
# speedup vs baseline: 1.2220x; 1.0416x over previous
"""Trainium2 Bass kernel for nn_EnhancedTextAttentionBlock (v2).

Self-contained: takes FULL inputs (as in reference.setup_inputs()), shards
across 8 NeuronCores internally, returns the FULL [2, 256, 48, 48] output.

Sharding: core c handles batch b = c // 4 and query-token block k = c % 4
(576 of the 2304 spatial tokens). K/V are computed for the full token set on
every core; a single SPMD program serves all 8 cores with no collectives.

Algebraic restructurings (exact, not approximations):
- pe depends only on (c, w): the 3x3 conv collapses to three 1-D convs (bf16).
- LayerNorms are FUSED into the projections: with kn = (tok - mu)*rs*g + b,
  kst = rs ⊙ (kwg^T tok - mu ⊗ kwg_sum) where kwg = kw*g.  The mu-correction
  rides as an extra K=1 matmul accumulation chunk; the per-token rs rides as
  the exp()'s per-partition scale (keys) or is folded into qst (queries).
  kn/qn are never materialized.
- The k-projection bias shifts every score of a query by a constant and
  cancels in softmax -> dropped.  v's LN beta and bias commute through the
  softmax-normalized attention -> folded into the output bias on host.
- Softmax denominators ride as a ones-column of v; av is normalized before
  a head-grouped output projection (4 heads packed into K=128).
- Softmax max-subtraction is skipped: |scores| < ~2 in fp32 exp range.
- Channel-wise token stats are computed as matmul COLUMNS (free-dim 1), so
  the whole stats scalar chain runs on [128, nchunk] tiles (~free).
"""
import math
import numpy as np

import concourse.bass as bass
import concourse.tile as tile
from concourse import bacc, mybir
from concourse.bass_utils import run_bass_kernel_spmd

import os as _os
F32 = mybir.dt.float32
BF16 = mybir.dt.bfloat16
R = mybir.dt.float32r
_PREC = _os.environ.get("KERNEL_PREC", "allr")
PROJ_DT = F32 if _PREC == "f32" else R      # q/k/v projections (moving = tok)
SCORES_DT = F32 if _PREC in ("f32", "sf32") else R
AV_DT = F32 if _PREC in ("f32", "af32") else BF16   # v / exp(scores) storage
OPROJ_DT = BF16 if _PREC == "obf" else F32
AF = mybir.ActivationFunctionType
OP = mybir.AluOpType

B, C, H, W, T = 2, 256, 48, 48, 512
NH, HD = 8, 32
S = H * W              # 2304 tokens
NQ = S // 4            # 576 q tokens per core
SCALE = HD ** -0.5
IT = 288               # q-tile (two per q block)
MC = 96                # epilogue chunk
MEGA = 1152            # k-side pipeline chunk (2 per S)
NCH = S // MEGA        # 2 mega chunks
NJM = MEGA // 128      # 9 key chunks per mega
EPS = 1e-5

# cvecs column indices (c-major [256, 1] vectors packed into one input)
CV_TMB1, CV_L1G, CV_L1B, CV_TMB2, CV_L2GN, CV_L2BN, CV_CONVB, CV_QBT = range(8)
# rowvecs (token-major prebroadcast [128, 256] rows)
RV_OB, RV_NOG = range(2)


def build_bass():
    nc = bacc.Bacc("TRN2", target_bir_lowering=False, debug=False,
                   enable_asserts=True, num_devices=8)
    di = {}

    def inp(name, shape, dt=F32):
        di[name] = nc.dram_tensor(name, shape, dt, kind="ExternalInput")
        return di[name]

    inp("xk", [C, S])
    inp("xq", [C, NQ])
    inp("xqres", [NQ, C])          # xq^T + no_b (host-folded)
    inp("text", [T, 1])
    inp("tmw1", [T, C])
    inp("tmw2", [C, C])
    inp("cvecs", [C, 8])
    inp("pe", [C, W])
    inp("w3b", [3, 768, C])
    inp("qwg", [C, C], PROJ_DT)    # [c, d] = q_w[d, c] * nq_g[c]
    inp("kwg", [C, C], PROJ_DT)
    inp("vwg", [C, C], PROJ_DT)    # [c, o] = v_w[o, c] * nkv_g[c]
    inp("owg", [128, 2, C], OPROJ_DT)
    inp("rowvecs", [128, 2, C])
    inp("vsn", [1, C], PROJ_DT)
    inp("sumrows", [1, 2, 2, 128], PROJ_DT)  # [0, r, dc, d]: r0 = -kwg_sum, r1 = -qwg_sum
    inp("gwg", [C, 1])
    inp("gvec", [MC, 2])           # bc cols: gwg_sum, gb_total
    inp("selmask", [128, 2, NQ])
    y = nc.dram_tensor("y", [NQ, C], F32, kind="ExternalOutput")
    if _os.environ.get("KERNEL_DBG") == "1":
        di["dbg_tok"] = nc.dram_tensor("dbg_tok", [128, 2, S], F32, kind="ExternalOutput")
        di["dbg_kst"] = nc.dram_tensor("dbg_kst", [128, 2, S], SCORES_DT, kind="ExternalOutput")
        di["dbg_gate"] = nc.dram_tensor("dbg_gate", [MC, 6], F32, kind="ExternalOutput")
        di["dbg_qst"] = nc.dram_tensor("dbg_qst", [128, 2, NQ], SCORES_DT, kind="ExternalOutput")
        di["dbg_pos"] = nc.dram_tensor("dbg_pos", [128, 2, 3, W], F32, kind="ExternalOutput")
        di["dbg_mod"] = nc.dram_tensor("dbg_mod", [128, 2], F32, kind="ExternalOutput")
        di["dbg_mlp"] = nc.dram_tensor("dbg_mlp", [128, 2, 3], F32, kind="ExternalOutput")

    with tile.TileContext(nc) as tc:
        _build_tile(nc, tc, di, y)
    nc.compile()
    return nc


MAGIC = 0x5f3759df
I32 = mybir.dt.int32


def _rsqrt_dve(nc, pool, out_ap, x_ap, shape, tag):
    """out = 1/sqrt(x) via bit-trick seed + 2 Newton steps (DVE only).

    x_ap must be an SBUF fp32 AP (read-only); out_ap may alias a target slice.
    """
    y = pool.tile(shape, F32, tag=f"{tag}_y")
    t = pool.tile(shape, F32, tag=f"{tag}_t")
    nc.vector.tensor_scalar(y.bitcast(I32)[:], x_ap.bitcast(I32), 1, None,
                            OP.logical_shift_right)
    nc.vector.tensor_scalar(y.bitcast(I32)[:], y.bitcast(I32)[:], -1, MAGIC,
                            OP.mult, OP.add)
    for _ in range(2):
        nc.vector.tensor_mul(t[:], y[:], y[:])
        nc.vector.tensor_tensor(t[:], t[:], x_ap, OP.mult)
        nc.vector.tensor_scalar(t[:], t[:], -0.5, 1.5, OP.mult, OP.add)
        nc.vector.tensor_tensor(y[:], y[:], t[:], OP.mult)
    nc.vector.tensor_copy(out_ap, y[:])


def _build_tile(nc, tc, di, y):
    with tc.tile_pool(name="cons", bufs=1) as cons:
        # ---- persistent small tiles ----
        ones_sb = cons.tile([128, 1], F32)
        nc.vector.memset(ones_sb[:], 1.0)
        eps1 = cons.tile([1, 1], F32)
        nc.vector.memset(eps1[:], EPS)
        epsc = cons.tile([128, 1], F32)
        nc.vector.memset(epsc[:], EPS)
        epsm = cons.tile([MC, 1], F32)
        nc.vector.memset(epsm[:], EPS)
        ones_r = cons.tile([128, 1], PROJ_DT)
        nc.vector.tensor_copy(ones_r[:], ones_sb[:])
        cv = cons.tile([128, 2, 8], F32)
        pe_sb = cons.tile([128, 2, W], F32)
        qw_sb = cons.tile([128, 2, C], PROJ_DT)
        kw_sb = cons.tile([128, 2, C], PROJ_DT)
        vw_sb = cons.tile([128, 2, C], PROJ_DT)
        ow_sb = cons.tile([128, 2, C], OPROJ_DT)
        rv_sb = cons.tile([128, 2, C], F32)
        vsn_sb = cons.tile([1, C], PROJ_DT)
        sr_sb = cons.tile([1, 2, 2, 128], PROJ_DT)
        gw_sb = cons.tile([128, 2, 1], F32)
        gv_sb = cons.tile([MC, 2], F32)
        sel_sb = cons.tile([128, 2, NQ], F32)
        posrow = cons.tile([128, 2, 3, W], F32)   # (cc, rowtype, w)
        dtop = cons.tile([128, 2, W], F32)
        dbot = cons.tile([128, 2, W], F32)
        # big persistent state
        tok = cons.tile([128, 2, S], F32)
        tok_r = cons.tile([128, 2, S], PROJ_DT)
        tokq = cons.tile([128, 2, NQ], F32)
        tokq_r = cons.tile([128, 2, NQ], PROJ_DT)
        kst = cons.tile([128, 2, S], SCORES_DT)
        qst = cons.tile([128, 2, NQ], SCORES_DT)
        v_tok = cons.tile([128, 18, NH, 33], AV_DT)
        murow = cons.tile([1, S], PROJ_DT)          # channel-SUM row (256*mu)
        rssc = cons.tile([128, 18], F32)        # rs(key) columns
        qmurow = cons.tile([1, NQ], PROJ_DT)    # q mean row
        qrsrow = cons.tile([1, NQ], F32)        # q rs row
        rsqbc = cons.tile([128, NQ], F32)       # SCALE * rs_q broadcast
        gate_sb = cons.tile([MC, 6], F32)
        xqres_sb = cons.tile([MC, 6, C], F32)

        with tc.tile_pool(name="work", bufs=2) as wk, \
             tc.tile_pool(name="worksm", bufs=3) as wks, \
             tc.tile_pool(name="pro", bufs=1) as pro, \
             tc.tile_pool(name="att2", bufs=2) as att2, \
             tc.tile_pool(name="atte", bufs=3) as atte, \
             tc.tile_pool(name="avn", bufs=2) as avnp, \
             tc.tile_pool(name="ps_s", bufs=2, space="PSUM") as ps_s, \
             tc.tile_pool(name="ps_av", bufs=1, space="PSUM") as ps_av, \
             tc.tile_pool(name="ps_m", bufs=2, space="PSUM") as ps_m:

            # ---- kick off the big input DMAs first ----
            # four parallel DMA queues: SP(sync) = MLP-critical, ACT = xk/xq,
            # DVE = conv/proj weights, Pool(SWDGE) = cold loads
            xk_re = di["xk"].rearrange("(c p) s -> p c s", p=128)
            nc.scalar.dma_start(out=tokq,
                                in_=di["xq"].rearrange("(c p) s -> p c s", p=128))
            for m in range(NCH):
                nc.scalar.dma_start(out=tok[:, :, m * MEGA:(m + 1) * MEGA],
                                    in_=xk_re[:, :, m * MEGA:(m + 1) * MEGA])
            nc.sync.dma_start(out=cv, in_=di["cvecs"].rearrange("(c p) v -> p c v", p=128))
            nc.sync.dma_start(out=pe_sb, in_=di["pe"].rearrange("(c p) w -> p c w", p=128))

            # ================= text modulation MLP (c-major) =================
            text_sb = pro.tile([128, 4, 1], F32, tag="text")
            nc.sync.dma_start(out=text_sb,
                              in_=di["text"].rearrange("(k p) o -> p k o", p=128))
            w1_sb = pro.tile([128, 4, C], F32, tag="w1")
            nc.sync.dma_start(out=w1_sb,
                              in_=di["tmw1"].rearrange("(k p) d -> p k d", p=128))
            w2_sb = pro.tile([128, 2, C], F32, tag="w2")
            nc.sync.dma_start(out=w2_sb,
                              in_=di["tmw2"].rearrange("(k p) d -> p k d", p=128))

            def cmajor_mlp_layer(x_col, w_sb, nkc, bias_col, tag):
                h_col = wks.tile([128, 2, 1], F32, tag=f"{tag}_h")
                for c2c in range(2):
                    h_ps = ps_m.tile([128, 1], F32, tag="ps")
                    for kc in range(nkc):
                        nc.tensor.matmul(
                            h_ps[:, :], w_sb[:, kc, c2c * 128:(c2c + 1) * 128],
                            x_col[:, kc, :], start=(kc == 0), stop=(kc == nkc - 1))
                    nc.scalar.activation(h_col[:, c2c, :], h_ps[:, :], AF.Identity,
                                         bias=bias_col[:, c2c, :])
                return h_col

            def cmajor_ln_stats(h_col, tag):
                sum_ps = ps_m.tile([1, 2], F32, tag="ps")
                hsq = wks.tile([128, 2, 1], F32, tag=f"{tag}_hsq")
                nc.vector.tensor_mul(hsq[:], h_col[:], h_col[:])
                for st, src_col in ((0, h_col), (1, hsq)):
                    for cc in range(2):
                        nc.tensor.matmul(sum_ps[:, st:st + 1], ones_sb[:],
                                         src_col[:, cc, :],
                                         start=(cc == 0), stop=(cc == 1))
                ms = wks.tile([1, 2], F32, tag=f"{tag}_ms")
                nc.vector.tensor_scalar_mul(ms[:], sum_ps[:, :], 1.0 / 256.0)
                var1 = wks.tile([1, 1], F32, tag=f"{tag}_var1")
                nc.vector.tensor_mul(var1[:], ms[:, 0:1], ms[:, 0:1])
                nc.vector.scalar_tensor_tensor(var1[:], ms[:, 1:2], EPS, var1[:],
                                               OP.add, OP.subtract)
                _rsqrt_dve(nc, wks, var1[:], var1[:], [1, 1], f"{tag}_rs")
                mu_b = wks.tile([128, 1], F32, tag=f"{tag}_mub")
                nc.gpsimd.partition_broadcast(mu_b[:], ms[:, 0:1])
                rs_b = wks.tile([128, 1], F32, tag=f"{tag}_rsb")
                nc.gpsimd.partition_broadcast(rs_b[:], var1[:])
                return mu_b, rs_b

            h1 = cmajor_mlp_layer(text_sb, w1_sb, 4, cv[:, :, CV_TMB1:CV_TMB1 + 1], "l1")
            mu_b, rs_b = cmajor_ln_stats(h1, "l1")
            h1n = wks.tile([128, 2, 1], F32, tag="h1n")
            for cc in range(2):
                nc.vector.scalar_tensor_tensor(
                    h1n[:, cc, :], h1[:, cc, :], mu_b[:, 0:1], rs_b[:],
                    OP.subtract, OP.mult)
                nc.scalar.activation(h1n[:, cc, :], h1n[:, cc, :], AF.Relu,
                                     bias=cv[:, cc, CV_L1B:CV_L1B + 1],
                                     scale=cv[:, cc, CV_L1G:CV_L1G + 1])
            h2 = cmajor_mlp_layer(h1n, w2_sb, 2, cv[:, :, CV_TMB2:CV_TMB2 + 1], "l2")
            if "dbg_mlp" in di:
                dmlp = cons.tile([128, 2, 3], F32)
                nc.vector.tensor_copy(dmlp[:, :, 0:1], h1[:])
                nc.vector.tensor_copy(dmlp[:, :, 1:2], h1n[:])
                nc.vector.tensor_copy(dmlp[:, :, 2:3], h2[:])
                nc.sync.dma_start(out=di["dbg_mlp"][:, :, :], in_=dmlp[:])
            mu2_b, rs2_b = cmajor_ln_stats(h2, "l2")
            mod = wks.tile([128, 2, 1], F32, tag="mod")
            state_mod = mod
            for cc in range(2):
                nc.vector.scalar_tensor_tensor(
                    mod[:, cc, :], h2[:, cc, :], mu2_b[:, 0:1], rs2_b[:],
                    OP.subtract, OP.mult)
                # sigmoid(z) = 1/(1+exp(-z)) via pre-negated g, b
                nc.scalar.activation(mod[:, cc, :], mod[:, cc, :], AF.Exp,
                                     bias=cv[:, cc, CV_L2BN:CV_L2BN + 1],
                                     scale=cv[:, cc, CV_L2GN:CV_L2GN + 1])
                nc.vector.tensor_scalar(mod[:, cc, :], mod[:, cc, :], 1.0, None, OP.add)
                nc.vector.reciprocal(mod[:, cc, :], mod[:, cc, :])

            # ---- conditional positional rows: 3 distinct conv rows (bf16) ----
            w3_sb = pro.tile([128, 3, 6, C], F32, tag="w3")
            nc.sync.dma_start(out=w3_sb,
                                in_=di["w3b"].rearrange("t (j p) m -> p t j m", p=128))
            nc.sync.dma_start(out=qw_sb, in_=di["qwg"].rearrange("(c p) d -> p c d", p=128))
            nc.sync.dma_start(out=gw_sb, in_=di["gwg"].rearrange("(c p) o -> p c o", p=128))
            nc.sync.dma_start(out=sel_sb, in_=di["selmask"][:, :, :])
            nc.sync.dma_start(out=kw_sb, in_=di["kwg"].rearrange("(c p) d -> p c d", p=128))
            nc.sync.dma_start(out=vw_sb, in_=di["vwg"].rearrange("(c p) d -> p c d", p=128))
            nc.gpsimd.dma_start(out=sr_sb, in_=di["sumrows"][:, :, :, :])
            nc.gpsimd.dma_start(out=gv_sb, in_=di["gvec"][:, :])
            nc.gpsimd.dma_start(out=vsn_sb, in_=di["vsn"][:, :])
            nc.gpsimd.dma_start(out=rv_sb, in_=di["rowvecs"][:, :, :])
            nc.gpsimd.dma_start(out=ow_sb, in_=di["owg"][:, :, :])
            nc.gpsimd.dma_start(out=xqres_sb,
                                in_=di["xqres"].rearrange("(k p) c -> p k c", p=MC))

            inrow = wks.tile([128, 2, W], F32, tag="inrow")
            for cc in range(2):
                nc.vector.tensor_scalar_mul(inrow[:, cc, :], pe_sb[:, cc, :],
                                            mod[:, cc, 0:1])
            im2 = wks.tile([128, 6, W], F32, tag="im2")
            nc.vector.memset(im2[:], 0.0)
            for kw in range(3):
                for cc in range(2):
                    j = kw * 2 + cc
                    if kw == 0:
                        nc.vector.tensor_copy(im2[:, j, 1:W], inrow[:, cc, 0:W - 1])
                    elif kw == 1:
                        nc.vector.tensor_copy(im2[:, j, :], inrow[:, cc, :])
                    else:
                        nc.vector.tensor_copy(im2[:, j, 0:W - 1], inrow[:, cc, 1:W])
            cps = ps_m.tile([128, 3, 2, W], F32, tag="ps")
            for t in range(3):
                for oc in range(2):
                    for j in range(6):
                        nc.tensor.matmul(cps[:, t, oc, :],
                                         w3_sb[:, t, j, oc * 128:(oc + 1) * 128],
                                         im2[:, j, :],
                                         start=(j == 0), stop=(j == 5))
            for cc in range(2):
                nc.vector.tensor_scalar(posrow[:, cc, :, :], cps[:, :, cc, :],
                                        cv[:, cc, CV_CONVB:CV_CONVB + 1], None, OP.add)
                nc.vector.tensor_sub(dtop[:, cc, :], posrow[:, cc, 0, :],
                                     posrow[:, cc, 1, :])
                nc.vector.tensor_sub(dbot[:, cc, :], posrow[:, cc, 2, :],
                                     posrow[:, cc, 1, :])

            # ================= q-side: tokens, stats, projections ============
            seltmp = pro.tile([128, NQ], F32, tag="seltmp")
            for cc in range(2):
                mid = posrow[:, cc, 1:2, :].to_broadcast([128, NQ // W, W])
                tq2 = tokq[:, cc, :].rearrange("p (h w) -> p h w", w=W)
                nc.vector.tensor_tensor(tq2, tq2, mid, OP.add)
                nc.vector.tensor_tensor(
                    seltmp[:].rearrange("p (h w) -> p h w", w=W),
                    sel_sb[:, 0, :].rearrange("p (h w) -> p h w", w=W),
                    dtop[:, cc, None, :].to_broadcast([128, NQ // W, W]), OP.mult)
                nc.vector.tensor_add(tokq[:, cc, :], tokq[:, cc, :], seltmp[:])
                nc.vector.tensor_tensor(
                    seltmp[:].rearrange("p (h w) -> p h w", w=W),
                    sel_sb[:, 1, :].rearrange("p (h w) -> p h w", w=W),
                    dbot[:, cc, None, :].to_broadcast([128, NQ // W, W]), OP.mult)
                nc.vector.tensor_add(tokq[:, cc, :], tokq[:, cc, :], seltmp[:])

            nc.scalar.dma_start(out=tokq_r, in_=tokq.bitcast(PROJ_DT)[:])

            # q stats in column form ([96, 6] chunks)
            sqq = pro.tile([128, 2, NQ], F32, tag="sqq")
            nc.vector.tensor_mul(sqq[:], tokq[:], tokq[:])
            scolq = ps_m.tile([MC, 6, 2], F32, tag="ps")
            for ch in range(6):
                for st, srct in ((0, tokq), (1, sqq)):
                    for cc in range(2):
                        nc.tensor.matmul(scolq[:, ch, st:st + 1],
                                         srct[:, cc, ch * MC:(ch + 1) * MC],
                                         ones_sb[:], start=(cc == 0), stop=(cc == 1))
            mrq = wks.tile([MC, 6, 2], F32, tag="mrq")   # [:,:,0]=mu  [:,:,1]=rs
            nc.vector.tensor_scalar_mul(mrq[:], scolq[:], 1.0 / 256.0)
            varq = wks.tile([MC, 6], F32, tag="varq")
            nc.vector.tensor_mul(varq[:], mrq[:, :, 0], mrq[:, :, 0])
            nc.vector.scalar_tensor_tensor(varq[:], mrq[:, :, 1], EPS, varq[:],
                                           OP.add, OP.subtract)
            _rsqrt_dve(nc, wks, mrq[:, :, 1], varq[:], [MC, 6], "rsq")
            # q-side row sums (mu row for the q-proj correction; rs_q row)
            sqrow = pro.tile([1, NQ], F32, tag="sqrow")
            for half in range(2):
                h0 = half * IT
                mrow_ps = ps_m.tile([1, IT], F32, tag="ps")
                for cc in range(2):
                    nc.tensor.matmul(mrow_ps[:, :], ones_r[:],
                                     tokq_r[:, cc, h0:h0 + IT],
                                     start=(cc == 0), stop=(cc == 1))
                nc.vector.tensor_copy(qmurow[:, h0:h0 + IT], mrow_ps[:, :])
                srow_ps = ps_m.tile([1, IT], F32, tag="ps")
                for cc in range(2):
                    nc.tensor.matmul(srow_ps[:, :], ones_sb[:],
                                     sqq[:, cc, h0:h0 + IT],
                                     start=(cc == 0), stop=(cc == 1))
                nc.vector.tensor_copy(sqrow[:, h0:h0 + IT], srow_ps[:, :])
            # rs_q row: rsqrt(sumsq/256 - (sum/256)^2 + eps)
            qsc = pro.tile([1, NQ], F32, tag="qsc")
            nc.vector.tensor_scalar_mul(qsc[:], qmurow.bitcast(F32)[:], 1.0 / 256.0)
            nc.vector.tensor_mul(qsc[:], qsc[:], qsc[:])
            nc.vector.scalar_tensor_tensor(qsc[:], sqrow[:], 1.0 / 256.0, qsc[:],
                                           OP.mult, OP.subtract)
            nc.vector.tensor_scalar(qsc[:], qsc[:], EPS, None, OP.add)
            _rsqrt_dve(nc, pro, qsc[:], qsc[:], [1, NQ], "qscr")
            nc.vector.tensor_scalar_mul(qsc[:], qsc[:], SCALE)
            nc.gpsimd.partition_broadcast(rsqbc[:], qsc[:])

            # q projection (fused LN): psum = qwg^T tokq - mu_q x qwg_sum
            for dc in range(2):
                for half in range(2):
                    h0 = half * IT
                    qp = ps_m.tile([128, IT], F32, tag="ps")
                    for cc in range(2):
                        nc.tensor.matmul(qp[:, :],
                                         qw_sb[:, cc, dc * 128:(dc + 1) * 128],
                                         tokq_r[:, cc, h0:h0 + IT],
                                         start=(cc == 0), stop=False)
                    nc.tensor.matmul(qp[:, :], sr_sb[0:1, 1, dc, :],
                                     qmurow[0:1, h0:h0 + IT],
                                     start=False, stop=True)
                    # qst = SCALE*rs_q (.) psum + qb_total x 1
                    tq = att2.tile([128, IT], F32, tag="tq")
                    nc.vector.tensor_mul(tq[:], qp[:, :], rsqbc[:, h0:h0 + IT])
                    nc.scalar.activation(qst[:, dc, h0:h0 + IT], tq[:], AF.Identity,
                                         bias=cv[:, dc, CV_QBT:CV_QBT + 1])

            # gate logits (fused LN), column form [96, 6]
            gcol = ps_m.tile([MC, 6], F32, tag="ps")
            for ch in range(6):
                for cc in range(2):
                    nc.tensor.matmul(gcol[:, ch:ch + 1],
                                     tokq[:, cc, ch * MC:(ch + 1) * MC],
                                     gw_sb[:, cc, :], start=(cc == 0), stop=(cc == 1))
            glog = wks.tile([MC, 6], F32, tag="glog")
            # glog = rs_q .* (raw - mu_q * gwg_sum) + gb_total
            nc.vector.tensor_scalar_mul(glog[:], mrq[:, :, 0], gv_sb[:, 0:1])
            nc.vector.tensor_sub(glog[:], gcol[:, :], glog[:])
            nc.vector.tensor_mul(glog[:], glog[:], mrq[:, :, 1])
            nc.vector.tensor_scalar(glog[:], glog[:], gv_sb[:, 1:2], None, OP.add)
            eg = wks.tile([MC, 6], F32, tag="eg")
            nc.scalar.activation(eg[:], glog[:], AF.Exp, scale=-1.0)
            nc.vector.tensor_scalar(gate_sb[:], eg[:], 1.0, None, OP.add)
            nc.vector.reciprocal(gate_sb[:], gate_sb[:])

            # ================= k-side mega-chunk pipeline ====================
            def kside_mega(m):
                t0 = m * MEGA
                nrows = MEGA // W  # 24
                for cc in range(2):
                    r0 = 0
                    if m == 0:   # image top row
                        nc.vector.tensor_add(tok[:, cc, 0:W], tok[:, cc, 0:W],
                                             posrow[:, cc, 0, :])
                        r0 = 1
                    r1 = nrows
                    if m == NCH - 1:  # image bottom row
                        nc.vector.tensor_add(tok[:, cc, t0 + MEGA - W:t0 + MEGA],
                                             tok[:, cc, t0 + MEGA - W:t0 + MEGA],
                                             posrow[:, cc, 2, :])
                        r1 = nrows - 1
                    a, b = t0 + r0 * W, t0 + r1 * W
                    mid = posrow[:, cc, 1:2, :].to_broadcast([128, r1 - r0, W])
                    tv = tok[:, cc, a:b].rearrange("p (h w) -> p h w", w=W)
                    nc.vector.tensor_tensor(tv, tv, mid, OP.add)
                # stats columns
                sq = pro.tile([128, 2, MEGA], F32, tag="sqk")
                nc.vector.tensor_mul(sq[:], tok[:, :, t0:t0 + MEGA],
                                     tok[:, :, t0:t0 + MEGA])
                nc.scalar.dma_start(out=tok_r[:, :, t0:t0 + MEGA],
                                    in_=tok.bitcast(PROJ_DT)[:, :, t0:t0 + MEGA])
                scol = ps_m.tile([128, NJM, 2], F32, tag="ps")
                for ch in range(NJM):
                    a = t0 + ch * 128
                    for cc in range(2):
                        nc.tensor.matmul(scol[:, ch, 0:1], tok[:, cc, a:a + 128],
                                         ones_sb[:], start=(cc == 0), stop=(cc == 1))
                    for cc in range(2):
                        nc.tensor.matmul(scol[:, ch, 1:2],
                                         sq[:, cc, ch * 128:(ch + 1) * 128],
                                         ones_sb[:], start=(cc == 0), stop=(cc == 1))
                mm = wks.tile([128, NJM, 2], F32, tag="mmk")
                nc.vector.tensor_scalar_mul(mm[:], scol[:], 1.0 / 256.0)
                var = wks.tile([128, NJM], F32, tag="vark")
                nc.vector.tensor_mul(var[:], mm[:, :, 0], mm[:, :, 0])
                nc.vector.scalar_tensor_tensor(var[:], mm[:, :, 1], EPS, var[:],
                                               OP.add, OP.subtract)
                rsk = rssc[:, m * NJM:(m + 1) * NJM]
                _rsqrt_dve(nc, wks, rsk, var[:], [128, NJM], "rsk")
                # channel-SUM rows (mu-correction moving operand)
                for quar in range(4):
                    a = t0 + quar * IT
                    mrow_ps = ps_m.tile([1, IT], F32, tag="ps")
                    for cc in range(2):
                        nc.tensor.matmul(mrow_ps[:, :], ones_r[:],
                                         tok_r[:, cc, a:a + IT],
                                         start=(cc == 0), stop=(cc == 1))
                    nc.vector.tensor_copy(murow[:, a:a + IT], mrow_ps[:, :])
                # k projection (fused LN, no bias; rs rides in the exp scale)
                for dc in range(2):
                    for quar in range(4):
                        a = t0 + quar * IT
                        kp = ps_m.tile([128, IT], F32, tag="ps")
                        for cc in range(2):
                            nc.tensor.matmul(kp[:, :],
                                             kw_sb[:, cc, dc * 128:(dc + 1) * 128],
                                             tok_r[:, cc, a:a + IT],
                                             start=(cc == 0), stop=False)
                        nc.tensor.matmul(kp[:, :], sr_sb[0:1, 0, dc, :],
                                         murow[0:1, a:a + IT],
                                         start=False, stop=True)
                        nc.scalar.copy(kst[:, dc, a:a + IT], kp[:, :])
                # v projection (fused LN; rs applied on the psum->sbuf copy)
                for ch in range(NJM):
                    jc = m * NJM + ch
                    a = t0 + ch * 128
                    vp = ps_m.tile([128, C], F32, tag="ps")
                    for cc in range(2):
                        nc.tensor.matmul(vp[:, :],
                                         tok_r[:, cc, a:a + 128],
                                         vw_sb[:, cc, :], start=(cc == 0), stop=False)
                    nc.tensor.matmul(vp[:, :], murow[0:1, a:a + 128],
                                     vsn_sb[0:1, :], start=False, stop=True)
                    nc.vector.tensor_scalar(
                        v_tok[:, jc, :, 0:32], vp[:, :].rearrange(
                            "p (h d) -> p h d", d=32),
                        rssc[:, jc:jc + 1], None, OP.mult)
                nc.vector.tensor_copy(
                    v_tok[:, m * NJM:(m + 1) * NJM, :, 32:33],
                    ones_sb[:, None, None, :].to_broadcast([128, NJM, NH, 1]))

            kside_mega(0)

            if "dbg_tok" in di:
                kside_mega(1)
                nc.sync.dma_start(out=di["dbg_tok"][:, :, :], in_=tok[:])
                nc.sync.dma_start(out=di["dbg_kst"][:, :, :], in_=kst[:])
                nc.sync.dma_start(out=di["dbg_gate"][:, :], in_=gate_sb[:])
                nc.sync.dma_start(out=di["dbg_qst"][:, :, :], in_=qst[:])
                nc.sync.dma_start(out=di["dbg_pos"][:, :, :, :], in_=posrow[:])
                nc.sync.dma_start(out=di["dbg_mod"][:, :], in_=state_mod[:, :, 0])

            # ================= attention + epilogue ==========================
            state = {"mod": state_mod}

            def attn_block(it, p, m):
                i0 = it * IT
                av_ps = state["av_ps"]
                for ch in range(NJM):
                    jc = m * NJM + ch
                    s_ps = ps_s.tile([128, 2, 512], F32, tag="sps")
                    for hh in range(2):
                        h = 2 * p + hh
                        dc, poff = h // 4, 32 * (h % 4)
                        nc.tensor.matmul(
                            s_ps[:, hh, 0:IT],
                            kst[poff:poff + 32, dc, jc * 128:(jc + 1) * 128],
                            qst[poff:poff + 32, dc, i0:i0 + IT],
                            start=True, stop=True, tile_position=(poff, 0))
                    e_sb = atte.tile([128, 2, IT], AV_DT, tag="esb")
                    nc.scalar.activation(e_sb[:, :, :], s_ps[:, :, 0:IT],
                                         AF.Exp, scale=rssc[:, jc:jc + 1])
                    for hh in range(2):
                        nc.tensor.matmul(
                            av_ps[:, hh, 0:IT], v_tok[:, jc, 2 * p + hh, :],
                            e_sb[:, hh, :],
                            start=(jc == 0), stop=(jc == 17))

            for it in range(2):
                av_n = avnp.tile([128, 2, IT], OPROJ_DT, tag="avn")
                for p in range(4):
                    av_ps = ps_av.tile([33, 2, 512], F32, tag="avps")
                    state["av_ps"] = av_ps
                    attn_block(it, p, 0)
                    if it == 0 and p == 0 and "dbg_tok" not in di:
                        kside_mega(1)
                    attn_block(it, p, 1)
                    # normalize: av_n[h] = av[h] * (1 / l[h])
                    av_ps = state["av_ps"]
                    r_sb = att2.tile([1, 2, IT], F32, tag="rsb")
                    nc.vector.reciprocal(r_sb[:], av_ps[32:33, :, 0:IT])
                    r_bc = att2.tile([32, 2, IT], F32, tag="rbc")
                    nc.gpsimd.partition_broadcast(r_bc[:], r_sb[:])
                    for hh in range(2):
                        h = 2 * p + hh
                        g, poff = h // 4, 32 * (h % 4)
                        nc.vector.tensor_tensor(
                            av_n[poff:poff + 32, g, :],
                            av_ps[0:32, hh, 0:IT], r_bc[:, hh, :], OP.mult)
                # output projection + epilogue per 96-token chunk
                for mc in range(3):
                    ch = it * 3 + mc
                    o_ps = ps_m.tile([MC, C], F32, tag="ps")
                    for g in range(2):
                        nc.tensor.matmul(o_ps[:, :],
                                         av_n[:, g, mc * MC:(mc + 1) * MC],
                                         ow_sb[:, g, :], start=(g == 0), stop=(g == 1))
                    og = wk.tile([MC, C], F32, tag="og")
                    nc.vector.tensor_add(og[:], o_ps[:, :], rv_sb[0:MC, RV_OB, :])
                    nc.vector.tensor_scalar_mul(og[:], og[:], gate_sb[:, ch:ch + 1])
                    stats = wks.tile([MC, nc.vector.BN_STATS_DIM], F32, tag="bst")
                    nc.vector.bn_stats(stats[:], og[:])
                    mv = wks.tile([MC, 2], F32, tag="bag")
                    nc.vector.bn_aggr(mv[:], stats[:])
                    rs2 = wks.tile([MC, 1], F32, tag="eprs")
                    nc.vector.tensor_scalar(rs2[:], mv[:, 1:2], EPS, None, OP.add)
                    _rsqrt_dve(nc, wks, rs2[:], rs2[:], [MC, 1], "eprsn")
                    rsn = wks.tile([MC, C], F32, tag="rsn")
                    nc.vector.tensor_scalar_mul(rsn[:], rv_sb[0:MC, RV_NOG, :], rs2[:])
                    t2 = wk.tile([MC, C], F32, tag="ept2")
                    nc.vector.scalar_tensor_tensor(
                        t2[:], og[:], mv[:, 0:1], rsn[:], OP.subtract, OP.mult)
                    nc.vector.tensor_add(t2[:], t2[:], xqres_sb[:, ch, :])
                    nc.sync.dma_start(
                        out=y.rearrange("(k p) c -> p k c", p=MC)[:, ch, :], in_=t2[:])


def _host_inputs(x, text_feature, tm_w1, tm_b1, tm_ln1_g, tm_ln1_b, tm_w2, tm_b2,
                 tm_ln2_g, tm_ln2_b, conv_w, conv_b, q_w, q_b, k_w, k_b, v_w, v_b,
                 o_w, o_b, gate_w, nq_g, nq_b, nkv_g, nkv_b, no_g, no_b):
    f = np.float32
    # pe table (depends only on (c, w); faithful to reference)
    div = np.exp(np.arange(C // 2, dtype=f) * (-math.log(10000.0) / (C // 2)))
    wpos = np.arange(W, dtype=f)
    s = np.sin(wpos[None, :] * div[:, None])
    c = np.cos(wpos[None, :] * div[:, None])
    pe = np.stack([s, c], axis=1).reshape(C, W).astype(f)
    # kh-collapsed conv kernels: top(kh 1,2), mid(all), bot(kh 0,1)
    w3 = np.stack([
        conv_w[:, :, 1, :] + conv_w[:, :, 2, :],
        conv_w.sum(axis=2),
        conv_w[:, :, 0, :] + conv_w[:, :, 1, :],
    ]).astype(f)                                  # [3, Cout, Cin, kw]
    w3 = w3.transpose(0, 3, 2, 1).reshape(3, 768, C)  # [(kw, cin), cout]
    w3b = np.ascontiguousarray(w3, dtype=f)

    # LN-fused projection weights
    qwg = np.ascontiguousarray(q_w.T * nq_g[:, None], dtype=f)   # [c, d]
    kwg = np.ascontiguousarray(k_w.T * nkv_g[:, None], dtype=f)
    vwg = np.ascontiguousarray(v_w.T * nkv_g[:, None], dtype=f)  # [c, o]
    qb_total = (q_b + q_w @ nq_b).astype(f)
    # correction rows pair with channel-SUM rows -> fold the 1/256 here
    sumrows = np.stack([-kwg.sum(axis=0) / 256.0, -qwg.sum(axis=0) / 256.0]) \
        .reshape(1, 2, 2, 128).astype(f)
    # v bias (incl LN beta) folds through softmax-normalized attention
    vb_total = (v_b + v_w @ nkv_b).astype(f)
    ob_eff = (o_b + vb_total @ o_w.T).astype(f)
    # gate
    gwg = np.ascontiguousarray((gate_w[0] * nq_g)[:, None], dtype=f)  # [C, 1]
    gvec = np.zeros((MC, 2), f)
    gvec[:, 0] = gwg.sum()
    gvec[:, 1] = gate_w[0] @ nq_b
    # head-grouped output projection: partition 32*(h%4)+d, group h//4
    owg = np.zeros((128, 2, C), f)
    for h in range(NH):
        owg[32 * (h % 4):32 * (h % 4) + 32, h // 4, :] = o_w[:, 32 * h:32 * h + 32].T
    if OPROJ_DT == BF16:
        import ml_dtypes
        owg = owg.astype(ml_dtypes.bfloat16)
    rowvecs = np.zeros((128, 2, C), f)
    rowvecs[:, RV_OB, :] = ob_eff[None, :]
    rowvecs[:, RV_NOG, :] = no_g[None, :]
    vsn = np.ascontiguousarray((-vwg.sum(axis=0) / 256.0)[None, :], dtype=f)
    cvecs = np.stack([
        tm_b1, tm_ln1_g, tm_ln1_b, tm_b2, -tm_ln2_g, -tm_ln2_b, conv_b, qb_total,
    ], axis=1).astype(f)                          # [256, 8]

    per_core = []
    for core in range(8):
        b, k = core // 4, core % 4
        xb = np.ascontiguousarray(x[b].reshape(C, S), dtype=f)
        xqc = np.ascontiguousarray(xb[:, NQ * k:NQ * (k + 1)])
        sel = np.zeros((128, 2, NQ), f)
        if k == 0:
            sel[:, 0, 0:W] = 1.0
        if k == 3:
            sel[:, 1, NQ - W:NQ] = 1.0
        per_core.append({
            "xk": xb,
            "xq": xqc,
            "xqres": np.ascontiguousarray(xqc.T + no_b[None, :]),
            "text": np.ascontiguousarray(text_feature[b][:, None], dtype=f),
            "tmw1": np.ascontiguousarray(tm_w1.T, dtype=f),
            "tmw2": np.ascontiguousarray(tm_w2.T, dtype=f),
            "cvecs": cvecs, "pe": pe, "w3b": w3b,
            "qwg": qwg, "kwg": kwg, "vwg": vwg, "owg": owg,
            "rowvecs": rowvecs, "sumrows": sumrows, "vsn": vsn,
            "gwg": gwg, "gvec": gvec, "selmask": sel,
        })
    return per_core


_NC_CACHE = {}


def get_nc():
    if "nc" not in _NC_CACHE:
        _NC_CACHE["nc"] = build_bass()
    return _NC_CACHE["nc"]


def kernel(**inputs):
    inputs = {k: np.asarray(v, dtype=np.float32) for k, v in inputs.items()}
    in_maps = _host_inputs(**inputs)
    nc = get_nc()
    res = run_bass_kernel_spmd(nc, in_maps, core_ids=list(range(8)))
    x = inputs["x"]
    out = np.empty((B, C, H, W), np.float32)
    for b in range(B):
        blocks = [res.results[4 * b + k]["y"] for k in range(4)]  # [NQ, C] each
        tok = np.concatenate(blocks, axis=0)                      # [S, C]
        out[b] = tok.T.reshape(C, H, W)
    return out
